# revision 24
# baseline (speedup 1.0000x reference)
"""NeuroSAT message-passing kernel for 8 Trainium2 NeuronCores (Bass/Tile).

Strategy
--------
The dense adjacency factors as A = D_row @ B @ D_col with B binary, so B
is carried in fp8 (1.0/0.0 exact in e4m3) as the *moving* matmul operand
against fp8 stationary message tiles (DoubleRow, K=256/instr); the degree
scalings are per-partition activation scales / free tensor_tensor
multiplies at PSUM eviction. The scaling-entangled final-layer MLP biases
are rank-1 (b3 outer scol/srow) and get added as precomputed outer-product
tiles on the (idle) Vector engine at eviction time.

Sharding (8 cores):
  - clauses: core k owns [2048k, 2048k+2048)
  - literals: core k owns [512k, 512k+512) u [4096+512k, 4096+512k+512)
    (a positive block and its negation block, so NeuroSAT's "flip" is a
    local slice swap instead of a cross-core exchange)

v3 (latency-stall rework, after v2's HBM rework):
  - 6 of the 8 b1 slabs (B[:, my clauses], lit-partitioned) live
    permanently in SBUF; only clause-chunk 3 of b1 plus all of b2 stream
    per round (~24MB/round vs 34 when streaming everything).
  - 4 collectives per round (2 lit-side, 2 clause-side AllGathers), each
    kicked the moment its half of the messages exists and consumed
    group-major on the other side, so the ~10us-floor CC ops pipeline
    under dense matmul work.
  - The serial LSTM/MLP chains (PSUM evict -> gates -> cell -> message
    MLP, mostly ACT/DVE latency) are emitted staggered one chunk behind
    the dense B-contraction so the in-order PE queue never waits on them;
    evictions are hoisted right after each accumulation closes. This keeps
    the PE continuously busy, which also keeps the HAM clock-gate at
    2.4GHz (idle gaps re-throttle it to 1.2GHz for ~3.4us).
"""
import sys

sys.path.insert(0, "/opt/trn_rl_repo")

import numpy as np
import ml_dtypes

import concourse.bass as bass
import concourse.mybir as mybir
import concourse.tile as tile
from concourse import bacc
from concourse import bass_utils

dt = mybir.dt
AF = mybir.ActivationFunctionType
ALU = mybir.AluOpType
bf16 = ml_dtypes.bfloat16
f8 = ml_dtypes.float8_e4m3

NCORES = 8
D = 128
NL_TOT, NCL_TOT, NV = 8192, 16384, 4096
NL = NL_TOT // NCORES      # 1024 lits per core
NCL = NCL_TOT // NCORES    # 2048 clauses per core
FP8_ONE = 0x38             # bit pattern of 1.0 in float8_e4m3
GAIN = np.float32(128.0)   # power-of-2 pre-scale keeping fp8 messages normal-range


# ---------------------------------------------------------------------------
# device program
# ---------------------------------------------------------------------------

def build_program(rounds: int):
    nc = bacc.Bacc("TRN2", target_bir_lowering=False, debug=False,
                   num_devices=NCORES)

    def inp(name, shape, dty):
        return nc.dram_tensor(name, list(shape), dty, kind="ExternalInput")

    # b1[nn, h]: slab of 32 k-tiles [128p, 32tt, 512c] covering clause chunk
    #            nn, lit-tile phase h (tt = 4k + jj, global tile t = 8k+4h+jj)
    # b2[nnl, kk, g]: sub-slab of 4 DR pairs [128p, 4q, 2, 512l]; lit
    #            out-chunk nnl, source core kk, clause half g. Pair (kk, g,
    #            q4) covers global clause tiles (16kk + 8g + 2*q4, +1).
    b1 = inp("b1", [4, 2, 128, 32 * 512], dt.float8e4)
    b2 = inp("b2", [2, 8, 2, 128, 4 * 1024], dt.float8e4)
    w = {}
    for p in ("lm", "cm", "lv"):
        for l in ("w1t", "w2t", "w3t"):
            shape = [128, 1] if (p, l) == ("lv", "w3t") else [128, 128]
            w[f"{p}_{l}"] = inp(f"{p}_{l}", shape, dt.bfloat16)
        for l in ("b1", "b2"):
            w[f"{p}_{l}"] = inp(f"{p}_{l}", [128, 1], dt.float32)
    cu_wt = inp("cu_wt", [128, 512], dt.bfloat16)      # cu_wih.T
    cu_ut = inp("cu_ut", [128, 512], dt.bfloat16)      # cu_whh.T
    cu_b = inp("cu_b", [128, 4], dt.float32)
    lu_wcl = inp("lu_wcl", [128, 512], dt.bfloat16)    # lu_wih[:, :128].T
    lu_wfl = inp("lu_wfl", [128, 512], dt.bfloat16)    # lu_wih[:, 128:].T
    lu_ut = inp("lu_ut", [128, 512], dt.bfloat16)      # lu_whh.T
    lu_b = inp("lu_b", [128, 4], dt.float32)
    korr_c = inp("korr_c", [128, NCL], dt.bfloat16)    # lm_b3 outer scol
    korr_l = inp("korr_l", [128, NL], dt.bfloat16)     # cm_b3 outer srow
    colb = inp("colb", [128, NCL], dt.bfloat16)        # col/GAIN bcast over parts
    rowb = inp("rowb", [128, NL], dt.bfloat16)         # row/GAIN bcast over parts
    rowsc = inp("rowsc", [128, 8], dt.float32)         # GAIN*row, per lit-tile col
    colsc = inp("colsc", [128, 16], dt.float32)        # GAIN*col, per clause-tile
    lh0 = inp("lh0", [128, NL], dt.bfloat16)
    ch0 = inp("ch0", [128, NCL], dt.bfloat16)

    vote_out = nc.dram_tensor("vote", [1, NL], dt.float32, kind="ExternalOutput")
    ag1_out = [nc.dram_tensor(f"ag1_out{h}", [NCORES, 128, 512], dt.float8e4,
                              addr_space="Shared") for h in range(2)]
    ag2_out = nc.dram_tensor("ag2_out", [NCORES, 128, 2048], dt.float8e4,
                             addr_space="Shared")
    rg = [list(range(NCORES))]

    with tile.TileContext(nc) as tc:
        with (
            tc.tile_pool(name="const", bufs=1) as cp,
            tc.tile_pool(name="state", bufs=1) as sp,
            tc.tile_pool(name="work", bufs=1) as wp,
            tc.tile_pool(name="chunk", bufs=2) as kp,
            tc.tile_pool(name="bstream", bufs=2) as bp,
            tc.tile_pool(name="psd", bufs=1, space="PSUM") as psd,
            tc.tile_pool(name="psg", bufs=1, space="PSUM") as psg,
            tc.tile_pool(name="psm", bufs=2, space="PSUM") as psm,
            tc.tile_pool(name="dram", bufs=1, space="DRAM") as dp,
        ):
            # ---- constants into SBUF ----
            C = {}
            for name, t in [
                ("cu_wt", cu_wt), ("cu_ut", cu_ut), ("lu_wcl", lu_wcl),
                ("lu_wfl", lu_wfl), ("lu_ut", lu_ut),
            ]:
                C[name] = cp.tile([128, 512], dt.bfloat16, name=name)
                nc.scalar.dma_start(out=C[name], in_=t.ap())
            for p in ("lm", "cm", "lv"):
                for l in ("w1t", "w2t", "w3t"):
                    shape = [128, 1] if (p, l) == ("lv", "w3t") else [128, 128]
                    C[f"{p}_{l}"] = cp.tile(shape, dt.bfloat16, name=f"{p}_{l}")
                    nc.scalar.dma_start(out=C[f"{p}_{l}"], in_=w[f"{p}_{l}"].ap())
                for l in ("b1", "b2"):
                    C[f"{p}_{l}"] = cp.tile([128, 1], dt.float32, name=f"{p}_{l}")
                    nc.scalar.dma_start(out=C[f"{p}_{l}"], in_=w[f"{p}_{l}"].ap())
            for name, t, shape, dty in [
                ("cu_b", cu_b, [128, 4], dt.float32),
                ("lu_b", lu_b, [128, 4], dt.float32),
                ("korr_c", korr_c, [128, NCL], dt.bfloat16),
                ("korr_l", korr_l, [128, NL], dt.bfloat16),
                ("colb", colb, [128, NCL], dt.bfloat16),
                ("rowb", rowb, [128, NL], dt.bfloat16),
                ("rowsc", rowsc, [128, 8], dt.float32),
                ("colsc", colsc, [128, 16], dt.float32),
            ]:
                C[name] = cp.tile(shape, dty, name=name)
                nc.scalar.dma_start(out=C[name], in_=t.ap())

            # ---- states ----
            Lh_pp = [sp.tile([128, NL], dt.bfloat16, name="Lh_a"),
                     sp.tile([128, NL], dt.bfloat16, name="Lh_b")]
            Ch = sp.tile([128, NCL], dt.bfloat16, name="Ch")
            Lc = sp.tile([128, NL], dt.float32, name="Lc")
            Cc = sp.tile([128, NCL], dt.float32, name="Cc")
            nc.scalar.dma_start(out=Lh_pp[0], in_=lh0.ap())
            nc.scalar.dma_start(out=Ch, in_=ch0.ap())
            nc.vector.memset(Lc, 0.0)
            nc.vector.memset(Cc, 0.0)

            # ---- resident B slices: 3 b1 slabs + source-cores 0-2 of b2
            # (48KB/part each); the rest streams per round, phase-balanced ----
            B1RES = ((0, 0), (1, 0), (2, 0))
            b1res = {}
            for nn, h in B1RES:
                b1res[nn, h] = cp.tile([128, 32 * 512], dt.float8e4,
                                       name=f"b1res{nn}{h}")
                nc.sync.dma_start(out=b1res[nn, h], in_=b1.ap()[nn, h])
            NRES2 = 3                  # resident source-cores of b2
            b2res = {}
            for nnl in range(2):
                b2res[nnl] = cp.tile([128, NRES2 * 2 * 4096], dt.float8e4,
                                     name=f"b2res{nnl}")
                nc.scalar.dma_start(
                    out=b2res[nnl].rearrange("p (k e) -> p k e", e=4096),
                    in_=b2.ap()[nnl][0:NRES2]
                    .rearrange("k g p e -> p (k g) e"))

            # ---- round-persistent work tiles ----
            lpre_full = wp.tile([128, NL_TOT], dt.float8e4, name="lpre_full")
            ag1_in = [dp.tile([128, 512], dt.float8e4, name=f"ag1_in{h}")
                      for h in range(2)]
            ag2_in = dp.tile([128, 2048], dt.float8e4, name="ag2_in")

            def msg_mlp(which, nn, src_sl, Lh_src=None):
                """3-layer message MLP for one 512 chunk -> img tile + stage."""
                pfx = "lm" if which == "l" else "cm"
                h1 = kp.tile([128, 512], dt.bfloat16, tag="h1",
                             name=f"h1{which}_{nn}")
                h2 = kp.tile([128, 512], dt.bfloat16, tag="h2",
                             name=f"h2{which}_{nn}")
                img = kp.tile([128, 512], dt.float8e4, tag="img",
                              name=f"img{which}_{nn}")
                src = Lh_src[:, src_sl] if which == "l" else Ch[:, src_sl]
                ps = psm.tile([128, 512], dt.float32, tag="m",
                              name=f"{pfx}1_{nn}")
                nc.tensor.matmul(ps, C[f"{pfx}_w1t"], src, start=True, stop=True)
                nc.scalar.activation(h1, ps, AF.Relu, bias=C[f"{pfx}_b1"])
                ps = psm.tile([128, 512], dt.float32, tag="m",
                              name=f"{pfx}2_{nn}")
                nc.tensor.matmul(ps, C[f"{pfx}_w2t"], h1, start=True, stop=True)
                nc.scalar.activation(h2, ps, AF.Relu, bias=C[f"{pfx}_b2"])
                scn = "rowsc" if which == "l" else "colsc"
                for j in range(4):
                    ps = psm.tile([128, 128], dt.float32, tag="m",
                                  name=f"{pfx}3_{nn}_{j}")
                    nc.tensor.matmul(ps, h2[:, 128 * j:128 * (j + 1)],
                                     C[f"{pfx}_w3t"], start=True, stop=True)
                    nc.scalar.activation(img[:, 128 * j:128 * (j + 1)], ps,
                                         AF.Copy,
                                         scale=C[scn][:, 4 * nn + j:
                                                      4 * nn + j + 1])
                if which == "l":
                    nc.scalar.dma_start(out=ag1_in[nn], in_=img)
                    nc.gpsimd.collective_compute(
                        "AllGather", ALU.bypass, replica_groups=rg,
                        ins=[ag1_in[nn].opt()], outs=[ag1_out[nn].ap().opt()])
                else:
                    nc.scalar.dma_start(
                        out=ag2_in[:, 512 * nn:512 * (nn + 1)], in_=img)
                    if nn == 3:
                        nc.gpsimd.collective_compute(
                            "AllGather", ALU.bypass, replica_groups=rg,
                            ins=[ag2_in.opt()],
                            outs=[ag2_out.ap().opt()])

            def land_ag1(h):
                # gathered group h -> lpre_full tiles t = 8kk + 4h + j
                dst = lpre_full.rearrange("p (k e) -> p k e", e=1024)[
                    :, :, 512 * h:512 * (h + 1)]
                nc.gpsimd.dma_start(out=dst, in_=ag1_out[h].ap()
                                    .rearrange("k p e -> p k e"))

            def land_ag2(q, r):
                """Land source-cores {2q, 2q+1} of the gathered cpre
                (32 clause-tiles: [kk_loc(2), g(2), q4(4), e(2)] order)."""
                cg = bp.tile([128, 32 * 128], dt.float8e4, tag="cgrp",
                             bufs=2, name=f"cgrp_{r}_{q}")
                nc.gpsimd.dma_start(
                    out=cg.rearrange("p (k e) -> p k e", e=2048),
                    in_=ag2_out.ap()[2 * q:2 * (q + 1)]
                    .rearrange("k p e -> p k e"))
                return cg

            def evict(which, cc, acc_ps):
                """PSUM -> scaled+bias-corrected message chunk (DVE only)."""
                sl = slice(512 * cc, 512 * (cc + 1))
                scale = C["colb"] if which == "c" else C["rowb"]
                korr = C["korr_c"] if which == "c" else C["korr_l"]
                msg = kp.tile([128, 512], dt.bfloat16, tag="msg", bufs=2,
                              name=f"msg_{which}_{cc}")
                nc.vector.tensor_tensor(out=msg, in0=acc_ps,
                                        in1=scale[:, sl], op=ALU.mult)
                nc.vector.tensor_tensor(out=msg, in0=msg, in1=korr[:, sl],
                                        op=ALU.add)
                return msg

            def lstm_rest(which, cc, msg, Lh_src=None, Lh_dst=None):
                """LSTM gates + state update for one 512-col chunk."""
                sl = slice(512 * cc, 512 * (cc + 1))
                gts = []
                for g in range(4):
                    gs = slice(128 * g, 128 * (g + 1))
                    ps = psg.tile([128, 512], dt.float32, tag=f"g{g % 2}",
                                  name=f"ps_{which}_{cc}_{g}")
                    if which == "c":
                        nc.tensor.matmul(ps, C["cu_wt"][:, gs], msg,
                                         start=True, stop=False,
                                         skip_group_check=True)
                        nc.tensor.matmul(ps, C["cu_ut"][:, gs], Ch[:, sl],
                                         start=False, stop=True,
                                         skip_group_check=True)
                        bias = C["cu_b"][:, g:g + 1]
                    else:
                        flip_sl = slice(512 * (1 - cc), 512 * (2 - cc))
                        nc.tensor.matmul(ps, C["lu_wcl"][:, gs], msg,
                                         start=True, stop=False,
                                         skip_group_check=True)
                        nc.tensor.matmul(ps, C["lu_wfl"][:, gs],
                                         Lh_src[:, flip_sl],
                                         start=False, stop=False,
                                         skip_group_check=True)
                        nc.tensor.matmul(ps, C["lu_ut"][:, gs], Lh_src[:, sl],
                                         start=False, stop=True,
                                         skip_group_check=True)
                        bias = C["lu_b"][:, g:g + 1]
                    gt = kp.tile([128, 512], dt.bfloat16, tag=f"gate{g}",
                                 bufs=1, name=f"gt_{which}_{cc}_{g}")
                    nc.scalar.activation(gt, ps,
                                         AF.Tanh if g == 2 else AF.Sigmoid,
                                         bias=bias)
                    gts.append(gt)
                cell = Cc if which == "c" else Lc
                hout = Ch if which == "c" else Lh_dst
                t1 = kp.tile([128, 512], dt.float32, tag="t1", bufs=1,
                             name=f"t1_{which}_{cc}")
                t2 = kp.tile([128, 512], dt.bfloat16, tag="t2", bufs=1,
                             name=f"t2_{which}_{cc}")
                nc.vector.tensor_tensor(out=t1, in0=gts[1], in1=cell[:, sl],
                                        op=ALU.mult)
                nc.vector.tensor_tensor(out=t2, in0=gts[0], in1=gts[2],
                                        op=ALU.mult)
                nc.vector.tensor_tensor(out=cell[:, sl], in0=t1, in1=t2,
                                        op=ALU.add)
                t3 = kp.tile([128, 512], dt.float32, tag="t1", bufs=1,
                             name=f"t3_{which}_{cc}")
                nc.scalar.activation(t3, cell[:, sl], AF.Tanh)
                nc.vector.tensor_tensor(out=hout[:, sl], in0=gts[3], in1=t3,
                                        op=ALU.mult)

            def b1_prefetch(r):
                """Kick streamed-b1 half-slab DMAs for round r (sync queue)."""
                tiles = {}
                for h in range(2):
                    for nn in range(4):
                        if (nn, h) in b1res:
                            continue
                        for s2 in range(2):
                            t = bp.tile([128, 16 * 512], dt.float8e4,
                                        tag="b1s", bufs=2,
                                        name=f"b1s_{r}_{nn}_{h}_{s2}")
                            nc.sync.dma_start(
                                out=t, in_=b1.ap()[nn, h][:, 16 * 512 * s2:
                                                          16 * 512 * (s2 + 1)])
                            tiles[nn, h, s2] = t
                return tiles

            def mm1_block(nn, h, b1tiles, first, last):
                """16 DoubleRow MMs: dir-1 k-phase h for clause chunk nn."""
                for s2 in range(2):
                    bseg = (b1res[nn, h][:, 16 * 512 * s2:16 * 512 * (s2 + 1)]
                            if (nn, h) in b1res else b1tiles[nn, h, s2])
                    for ttp in range(8):
                        tt = 16 * s2 + 2 * ttp
                        t = 8 * (tt // 4) + 4 * h + tt % 4
                        lhsT = lpre_full[:, 128 * t:128 * (t + 2)] \
                            .rearrange("p (e d) -> p e d", e=2)
                        rhs = bseg[:, 1024 * ttp:1024 * (ttp + 1)] \
                            .rearrange("p (e c) -> p e c", e=2)
                        nc.tensor.matmul(
                            ps1[nn], lhsT, rhs,
                            start=(first and s2 == 0 and ttp == 0),
                            stop=(last and s2 == 1 and ttp == 7),
                            perf_mode=mybir.MatmulPerfMode.DoubleRow,
                            skip_group_check=True)

            def b2_prefetch(r):
                """Kick streamed-b2 DMAs for round r (scalar queue), one
                8KB tile per (nnl, source-core) covering both clause
                halves, issued in exact consumption order."""
                tiles = {}
                for kk in range(NRES2, 8):
                    for nnl in range(2):
                        t = bp.tile([128, 2 * 4096], dt.float8e4,
                                    tag="b2s", bufs=2,
                                    name=f"b2s_{r}_{nnl}_{kk}")
                        nc.scalar.dma_start(
                            out=t.rearrange("p (g e) -> p g e", e=4096),
                            in_=b2.ap()[nnl, kk]
                            .rearrange("g p e -> p g e"))
                        tiles[nnl, kk] = t
                return tiles

            def mm2_block(nnl, q, cg, b2tiles, first, last):
                """16 DoubleRow MMs: dir-2 source-quarter q for lit chunk
                nnl. cgrp position of pair (kk, g, q4):
                i = 16*(kk-2q) + 8*g + 2*q4."""
                for kk in (2 * q, 2 * q + 1):
                    for g in range(2):
                        off = 4096 * (2 * kk + g)
                        rhs_base = (b2res[nnl][:, off:off + 4096]
                                    if kk < NRES2 else
                                    b2tiles[nnl, kk][:, 4096 * g:
                                                     4096 * (g + 1)])
                        for q4 in range(4):
                            i = 16 * (kk - 2 * q) + 8 * g + 2 * q4
                            lhsT = cg[:, 128 * i:128 * (i + 2)] \
                                .rearrange("p (e d) -> p e d", e=2)
                            rhs = rhs_base[:, 1024 * q4:1024 * (q4 + 1)] \
                                .rearrange("p (e c) -> p e c", e=2)
                            nc.tensor.matmul(
                                ps2[nnl], lhsT, rhs,
                                start=(first and kk == 2 * q and g == 0
                                       and q4 == 0),
                                stop=(last and kk == 2 * q + 1 and g == 1
                                      and q4 == 3),
                                perf_mode=mybir.MatmulPerfMode.DoubleRow,
                                skip_group_check=True)

            # ---- prologue: L message of round 0 (kicks AG1_0, AG1_1) ----
            for nn in range(2):
                msg_mlp("l", nn, slice(512 * nn, 512 * (nn + 1)),
                        Lh_src=Lh_pp[0])

            for r in range(rounds):
                Lh = Lh_pp[r % 2]
                Lh_new = Lh_pp[(r + 1) % 2]

                # ===== dir-1 (A^T @ Lpre), phase-major; C side staggered one
                # chunk behind so its serial ACT/DVE chain hides under the
                # next chunk's dense MMs =====
                ps1 = [psd.tile([128, 512], dt.float32, tag=f"d{nn}",
                                name=f"ps1_{r}_{nn}") for nn in range(4)]
                b1t = b1_prefetch(r)
                b2t = b2_prefetch(r)
                cmsgs = [None] * 4
                for h in range(2):
                    land_ag1(h)
                    if h == 0:
                        for nn in range(4):
                            mm1_block(nn, 0, b1t, first=True, last=False)
                    else:
                        for nn in range(4):
                            mm1_block(nn, 1, b1t, first=False, last=True)
                            cmsgs[nn] = evict("c", nn, ps1[nn])
                            if nn >= 1:
                                cs = nn - 1
                                lstm_rest("c", cs, cmsgs[cs])
                                msg_mlp("c", cs, slice(512 * cs, 512 * (cs + 1)))
                        for cs in (3,):
                            lstm_rest("c", cs, cmsgs[cs])
                            msg_mlp("c", cs, slice(512 * cs, 512 * (cs + 1)))

                # ===== dir-2 (A @ Cpre) group-major + L side staggered =====
                ps2 = [psd.tile([128, 512], dt.float32, tag=f"d{nnl}",
                                name=f"ps2_{r}_{nnl}") for nnl in range(2)]
                for q in range(4):
                    cgrp = land_ag2(q, r)
                    if q < 3:
                        mm2_block(0, q, cgrp, b2t, first=(q == 0), last=False)
                        mm2_block(1, q, cgrp, b2t, first=(q == 0), last=False)
                    else:
                        mm2_block(0, 3, cgrp, b2t, first=False, last=True)
                        lmsg0 = evict("l", 0, ps2[0])
                        mm2_block(1, 3, cgrp, b2t, first=False, last=True)
                        lstm_rest("l", 0, lmsg0, Lh_src=Lh, Lh_dst=Lh_new)
                        if r < rounds - 1:
                            msg_mlp("l", 0, slice(0, 512), Lh_src=Lh_new)
                        lmsg1 = evict("l", 1, ps2[1])
                        lstm_rest("l", 1, lmsg1, Lh_src=Lh, Lh_dst=Lh_new)
                        if r < rounds - 1:
                            msg_mlp("l", 1, slice(512, 1024), Lh_src=Lh_new)

            # ===== vote MLP (bias of last layer added host-side) =====
            Lh_fin = Lh_pp[rounds % 2]
            vote_sb = wp.tile([1, NL], dt.float32, name="vote_sb")
            for nn in range(2):
                sl = slice(512 * nn, 512 * (nn + 1))
                h1 = kp.tile([128, 512], dt.bfloat16, tag="h1", name=f"vh1_{nn}")
                h2 = kp.tile([128, 512], dt.bfloat16, tag="h2", name=f"vh2_{nn}")
                ps = psm.tile([128, 512], dt.float32, tag="m", name=f"v1_{nn}")
                nc.tensor.matmul(ps, C["lv_w1t"], Lh_fin[:, sl],
                                 start=True, stop=True)
                nc.scalar.activation(h1, ps, AF.Relu, bias=C["lv_b1"])
                ps = psm.tile([128, 512], dt.float32, tag="m", name=f"v2_{nn}")
                nc.tensor.matmul(ps, C["lv_w2t"], h1,
                                 start=True, stop=True)
                nc.scalar.activation(h2, ps, AF.Relu, bias=C["lv_b2"])
                ps = psm.tile([1, 512], dt.float32, tag="m", name=f"v3_{nn}")
                nc.tensor.matmul(ps, C["lv_w3t"], h2,
                                 start=True, stop=True)
                nc.scalar.activation(vote_sb[0:1, sl], ps, AF.Copy)
            nc.scalar.dma_start(out=vote_out.ap(), in_=vote_sb)

    nc.compile()
    return nc


# ---------------------------------------------------------------------------
# host-side input preparation
# ---------------------------------------------------------------------------

def prep_inputs(inputs):
    g = {k: np.asarray(v) for k, v in inputs.items()}
    lit_idx = g["lit_idx"].astype(np.int64)
    clause_idx = g["clause_idx"].astype(np.int64)

    B = np.zeros((NL_TOT, NCL_TOT), np.bool_)
    B[lit_idx, clause_idx] = True
    degc = B.sum(0).astype(np.float32)
    degl = B.sum(1).astype(np.float32)
    col = (np.float32(1.0) / (np.sqrt(degc) + np.float32(1e-6))).astype(np.float32)
    row = (np.float32(1.0) / (np.sqrt(degl) + np.float32(1e-6))).astype(np.float32)
    # degree-0 rows/cols of A are structurally zero: clamp their scales so the
    # gained fp8 messages stay finite (mathematically identical result)
    col = np.where(degc > 0, col, np.float32(0)).astype(np.float32)
    row = np.where(degl > 0, row, np.float32(0)).astype(np.float32)

    # permuted lit order: core k <- [512k..512k+512) u [4096+512k..4096+512k+512)
    lit_order = np.concatenate(
        [np.concatenate([np.arange(512 * k, 512 * (k + 1)),
                         NV + np.arange(512 * k, 512 * (k + 1))])
         for k in range(NCORES)])
    Bu = B.astype(np.uint8) * FP8_ONE
    Bp = Bu[lit_order]                      # [8192, 16384] permuted rows
    row_p = row[lit_order]

    Bf32 = B.astype(np.float32)
    s_c = row @ Bf32                        # [NCL_TOT]
    scol_full = (col * s_c).astype(np.float32)
    s_l = Bf32 @ col
    srow_full = ((row * s_l).astype(np.float32))[lit_order]

    lm_b3 = np.asarray(g["lm_b3"], np.float32)
    cm_b3 = np.asarray(g["cm_b3"], np.float32)

    def b(x):
        return np.ascontiguousarray(np.asarray(x, np.float32)).astype(bf16)

    common = {
        "lm_w1t": b(g["lm_w1"].T), "lm_w2t": b(g["lm_w2"].T), "lm_w3t": b(g["lm_w3"].T),
        "cm_w1t": b(g["cm_w1"].T), "cm_w2t": b(g["cm_w2"].T), "cm_w3t": b(g["cm_w3"].T),
        "lv_w1t": b(g["lv_w1"].T), "lv_w2t": b(g["lv_w2"].T), "lv_w3t": b(g["lv_w3"].T),
        "lm_b1": np.asarray(g["lm_b1"], np.float32).reshape(128, 1),
        "lm_b2": np.asarray(g["lm_b2"], np.float32).reshape(128, 1),
        "cm_b1": np.asarray(g["cm_b1"], np.float32).reshape(128, 1),
        "cm_b2": np.asarray(g["cm_b2"], np.float32).reshape(128, 1),
        "lv_b1": np.asarray(g["lv_b1"], np.float32).reshape(128, 1),
        "lv_b2": np.asarray(g["lv_b2"], np.float32).reshape(128, 1),
        "cu_wt": b(g["cu_wih"].T), "cu_ut": b(g["cu_whh"].T),
        "lu_wcl": b(g["lu_wih"][:, :D].T), "lu_wfl": b(g["lu_wih"][:, D:].T),
        "lu_ut": b(g["lu_whh"].T),
        "cu_b": np.asarray(g["cu_bih"] + g["cu_bhh"], np.float32).reshape(4, 128).T.copy(),
        "lu_b": np.asarray(g["lu_bih"] + g["lu_bhh"], np.float32).reshape(4, 128).T.copy(),
        "lh0": np.ascontiguousarray(np.broadcast_to(
            np.asarray(g["L_init_w"][:, 0] + g["L_init_b"], np.float32)[:, None],
            (128, NL))).astype(bf16),
        "ch0": np.ascontiguousarray(np.broadcast_to(
            np.asarray(g["C_init_w"][:, 0] + g["C_init_b"], np.float32)[:, None],
            (128, NCL))).astype(bf16),
    }

    in_maps = []
    for k in range(NCORES):
        lsl = slice(NL * k, NL * (k + 1))
        csl = slice(NCL * k, NCL * (k + 1))
        # b1: Bp[:, csl] is [t(64)*128p rows, nn(4)*512c cols]
        #     [kk(8), jh(2), jj(4), p, nn, c] -> [nn, jh, p, kk, jj, c]
        X = Bp[:, csl].reshape(8, 2, 4, 128, 4, 512)
        b1k = np.ascontiguousarray(X.transpose(4, 1, 3, 0, 2, 5)).reshape(
            4, 2, 128, 32 * 512).view(f8)
        # b2: Bp[lsl, :].T is [T(128)*128p clause rows, nnl(2)*512l cols]
        #     T = 16*kk + 8*g + 2*q4 + e
        Y = Bp[lsl, :].T.reshape(8, 2, 4, 2, 128, 2, 512)
        # axes: kk, g, q4, e, p, nnl, l -> nnl, kk, g, p, q4, e, l
        b2k = np.ascontiguousarray(Y.transpose(5, 0, 1, 4, 2, 3, 6)).reshape(
            2, 8, 2, 128, 4 * 1024).view(f8)
        m = dict(common)
        m.update({
            "b1": b1k,
            "b2": b2k,
            "korr_c": np.ascontiguousarray(
                lm_b3[:, None] * scol_full[None, csl]).astype(bf16),
            "korr_l": np.ascontiguousarray(
                cm_b3[:, None] * srow_full[None, lsl]).astype(bf16),
            "colb": np.ascontiguousarray(
                np.broadcast_to(col[csl][None, :] / GAIN, (128, NCL))).astype(bf16),
            "rowb": np.ascontiguousarray(
                np.broadcast_to(row_p[lsl][None, :] / GAIN, (128, NL))).astype(bf16),
            "rowsc": np.ascontiguousarray(
                GAIN * row_p[lsl].reshape(8, 128).T).astype(np.float32),
            "colsc": np.ascontiguousarray(
                GAIN * col[csl].reshape(16, 128).T).astype(np.float32),
        })
        in_maps.append(m)
    return in_maps


def selfcheck_layouts(in_maps, lit_idx, clause_idx):
    """Random probes: device-layout b1/b2 entries vs the raw B matrix."""
    B = np.zeros((NL_TOT, NCL_TOT), np.uint8)
    B[lit_idx, clause_idx] = FP8_ONE
    lit_order = np.concatenate(
        [np.concatenate([np.arange(512 * k, 512 * (k + 1)),
                         NV + np.arange(512 * k, 512 * (k + 1))])
         for k in range(NCORES)])
    Bp = B[lit_order]
    rng = np.random.default_rng(1)
    for k in (0, 3):
        b1k = in_maps[k]["b1"].view(np.uint8).reshape(4, 2, 128, 32, 512)
        for _ in range(50):
            nn, h, p, tt, c = (rng.integers(4), rng.integers(2), rng.integers(128),
                               rng.integers(32), rng.integers(512))
            t = 8 * (tt // 4) + 4 * h + tt % 4
            want = Bp[128 * t + p, 2048 * k + 512 * nn + c]
            assert b1k[nn, h, p, tt, c] == want, (k, nn, h, p, tt, c)
        b2k = in_maps[k]["b2"].view(np.uint8).reshape(2, 8, 2, 128, 4, 2, 512)
        for _ in range(80):
            nnl, kk, gg, p, q4, e, l = (
                rng.integers(2), rng.integers(8), rng.integers(2),
                rng.integers(128), rng.integers(4), rng.integers(2),
                rng.integers(512))
            T = 16 * kk + 8 * gg + 2 * q4 + e
            want = Bp[1024 * k + 512 * nnl + l, 128 * T + p]
            assert b2k[nnl, kk, gg, p, q4, e, l] == want, (k, nnl, kk, gg, p, q4, e, l)


_PROGRAM_CACHE = {}


def _get_program(rounds):
    if rounds not in _PROGRAM_CACHE:
        _PROGRAM_CACHE[rounds] = build_program(rounds)
    return _PROGRAM_CACHE[rounds]


def run_device(inputs, trace=False, rounds=None, **kw):
    if rounds is None:
        rounds = int(inputs.get("n_rounds", 16))
    in_maps = prep_inputs(inputs)
    nc = _get_program(rounds)
    res = bass_utils.run_bass_kernel_spmd(
        nc, in_maps, core_ids=list(range(NCORES)), trace=trace, **kw)
    return res


def assemble_votes(res_results, lv_b3):
    votes = np.stack([np.asarray(res_results[k]["vote"]).reshape(NL)
                      for k in range(NCORES)])   # [8, 1024]
    vote = votes + np.float32(lv_b3)
    pos = vote[:, :512].reshape(NV)              # var v -> core v//512
    neg = vote[:, 512:].reshape(NV)
    vj = np.stack([pos, neg], axis=1)            # [4096, 2]
    return vj.reshape(32, -1).mean(axis=1).astype(np.float32)


def kernel(**inputs) -> np.ndarray:
    res = run_device(inputs)
    return assemble_votes(res.results, np.asarray(inputs["lv_b3"]).reshape(-1)[0])


# revision 26
# speedup vs baseline: 1.0347x; 1.0347x over previous
"""NeuroSAT message-passing kernel for 8 Trainium2 NeuronCores (Bass/Tile).

Strategy
--------
The dense adjacency factors as A = D_row @ B @ D_col with B binary, so B
is carried in fp8 (1.0/0.0 exact in e4m3) as the *moving* matmul operand
against fp8 stationary message tiles (DoubleRow, K=256/instr); the degree
scalings are per-partition activation scales / free tensor_tensor
multiplies at PSUM eviction. The scaling-entangled final-layer MLP biases
are rank-1 (b3 outer scol/srow) and get added as precomputed outer-product
tiles on the (idle) Vector engine at eviction time.

Sharding (8 cores):
  - clauses: core k owns [2048k, 2048k+2048)
  - literals: core k owns [512k, 512k+512) u [4096+512k, 4096+512k+512)
    (a positive block and its negation block, so NeuroSAT's "flip" is a
    local slice swap instead of a cross-core exchange)

v3 (latency-stall rework, after v2's HBM rework):
  - 6 of the 8 b1 slabs (B[:, my clauses], lit-partitioned) live
    permanently in SBUF; only clause-chunk 3 of b1 plus all of b2 stream
    per round (~24MB/round vs 34 when streaming everything).
  - 4 collectives per round (2 lit-side, 2 clause-side AllGathers), each
    kicked the moment its half of the messages exists and consumed
    group-major on the other side, so the ~10us-floor CC ops pipeline
    under dense matmul work.
  - The serial LSTM/MLP chains (PSUM evict -> gates -> cell -> message
    MLP, mostly ACT/DVE latency) are emitted staggered one chunk behind
    the dense B-contraction so the in-order PE queue never waits on them;
    evictions are hoisted right after each accumulation closes. This keeps
    the PE continuously busy, which also keeps the HAM clock-gate at
    2.4GHz (idle gaps re-throttle it to 1.2GHz for ~3.4us).
"""
import sys

sys.path.insert(0, "/opt/trn_rl_repo")

import numpy as np
import ml_dtypes

import concourse.bass as bass
import concourse.mybir as mybir
import concourse.tile as tile
from concourse import bacc
from concourse import bass_utils

dt = mybir.dt
AF = mybir.ActivationFunctionType
ALU = mybir.AluOpType
bf16 = ml_dtypes.bfloat16
f8 = ml_dtypes.float8_e4m3

NCORES = 8
D = 128
NL_TOT, NCL_TOT, NV = 8192, 16384, 4096
NL = NL_TOT // NCORES      # 1024 lits per core
NCL = NCL_TOT // NCORES    # 2048 clauses per core
FP8_ONE = 0x38             # bit pattern of 1.0 in float8_e4m3
GAIN = np.float32(128.0)   # power-of-2 pre-scale keeping fp8 messages normal-range


# ---------------------------------------------------------------------------
# device program
# ---------------------------------------------------------------------------

def build_program(rounds: int):
    nc = bacc.Bacc("TRN2", target_bir_lowering=False, debug=False,
                   num_devices=NCORES)

    def inp(name, shape, dty):
        return nc.dram_tensor(name, list(shape), dty, kind="ExternalInput")

    # b1[nn, h]: slab of 32 k-tiles [128p, 32tt, 512c] covering clause chunk
    #            nn, lit-tile phase h (tt = 4k + jj, global tile t = 8k+4h+jj)
    # b2[nnl, kk, g]: sub-slab of 4 DR pairs [128p, 4q, 2, 512l]; lit
    #            out-chunk nnl, source core kk, clause half g. Pair (kk, g,
    #            q4) covers global clause tiles (16kk + 8g + 2*q4, +1).
    b1 = inp("b1", [4, 2, 128, 32 * 512], dt.float8e4)
    b2 = inp("b2", [2, 8, 2, 128, 4 * 1024], dt.float8e4)
    w = {}
    for p in ("lm", "cm", "lv"):
        for l in ("w1t", "w2t", "w3t"):
            shape = [128, 1] if (p, l) == ("lv", "w3t") else [128, 128]
            w[f"{p}_{l}"] = inp(f"{p}_{l}", shape, dt.bfloat16)
        for l in ("b1", "b2"):
            w[f"{p}_{l}"] = inp(f"{p}_{l}", [128, 1], dt.float32)
    cu_wt = inp("cu_wt", [128, 512], dt.bfloat16)      # cu_wih.T
    cu_ut = inp("cu_ut", [128, 512], dt.bfloat16)      # cu_whh.T
    cu_b = inp("cu_b", [128, 4], dt.float32)
    lu_wcl = inp("lu_wcl", [128, 512], dt.bfloat16)    # lu_wih[:, :128].T
    lu_wfl = inp("lu_wfl", [128, 512], dt.bfloat16)    # lu_wih[:, 128:].T
    lu_ut = inp("lu_ut", [128, 512], dt.bfloat16)      # lu_whh.T
    lu_b = inp("lu_b", [128, 4], dt.float32)
    korr_c = inp("korr_c", [128, NCL], dt.bfloat16)    # lm_b3 outer scol
    korr_l = inp("korr_l", [128, NL], dt.bfloat16)     # cm_b3 outer srow
    colb = inp("colb", [128, NCL], dt.bfloat16)        # col/GAIN bcast over parts
    rowb = inp("rowb", [128, NL], dt.bfloat16)         # row/GAIN bcast over parts
    rowsc = inp("rowsc", [128, 8], dt.float32)         # GAIN*row, per lit-tile col
    colsc = inp("colsc", [128, 16], dt.float32)        # GAIN*col, per clause-tile
    lh0 = inp("lh0", [128, NL], dt.bfloat16)
    ch0 = inp("ch0", [128, NCL], dt.bfloat16)

    vote_out = nc.dram_tensor("vote", [1, NL], dt.float32, kind="ExternalOutput")
    ag1_out = [nc.dram_tensor(f"ag1_out{h}", [NCORES, 128, 512], dt.float8e4,
                              addr_space="Shared") for h in range(2)]
    ag2_out = [nc.dram_tensor(f"ag2_out{g}", [NCORES, 128, 1024], dt.float8e4,
                              addr_space="Shared") for g in range(2)]
    rg = [list(range(NCORES))]

    with tile.TileContext(nc) as tc:
        with (
            tc.tile_pool(name="const", bufs=1) as cp,
            tc.tile_pool(name="state", bufs=1) as sp,
            tc.tile_pool(name="work", bufs=1) as wp,
            tc.tile_pool(name="chunk", bufs=2) as kp,
            tc.tile_pool(name="bstream", bufs=2) as bp,
            tc.tile_pool(name="psd", bufs=1, space="PSUM") as psd,
            tc.tile_pool(name="psg", bufs=1, space="PSUM") as psg,
            tc.tile_pool(name="psm", bufs=2, space="PSUM") as psm,
            tc.tile_pool(name="dram", bufs=1, space="DRAM") as dp,
        ):
            # ---- constants into SBUF ----
            C = {}
            for name, t in [
                ("cu_wt", cu_wt), ("cu_ut", cu_ut), ("lu_wcl", lu_wcl),
                ("lu_wfl", lu_wfl), ("lu_ut", lu_ut),
            ]:
                C[name] = cp.tile([128, 512], dt.bfloat16, name=name)
                nc.scalar.dma_start(out=C[name], in_=t.ap())
            for p in ("lm", "cm", "lv"):
                for l in ("w1t", "w2t", "w3t"):
                    shape = [128, 1] if (p, l) == ("lv", "w3t") else [128, 128]
                    C[f"{p}_{l}"] = cp.tile(shape, dt.bfloat16, name=f"{p}_{l}")
                    nc.scalar.dma_start(out=C[f"{p}_{l}"], in_=w[f"{p}_{l}"].ap())
                for l in ("b1", "b2"):
                    C[f"{p}_{l}"] = cp.tile([128, 1], dt.float32, name=f"{p}_{l}")
                    nc.scalar.dma_start(out=C[f"{p}_{l}"], in_=w[f"{p}_{l}"].ap())
            for name, t, shape, dty in [
                ("cu_b", cu_b, [128, 4], dt.float32),
                ("lu_b", lu_b, [128, 4], dt.float32),
                ("korr_c", korr_c, [128, NCL], dt.bfloat16),
                ("korr_l", korr_l, [128, NL], dt.bfloat16),
                ("colb", colb, [128, NCL], dt.bfloat16),
                ("rowb", rowb, [128, NL], dt.bfloat16),
                ("rowsc", rowsc, [128, 8], dt.float32),
                ("colsc", colsc, [128, 16], dt.float32),
            ]:
                C[name] = cp.tile(shape, dty, name=name)
                nc.scalar.dma_start(out=C[name], in_=t.ap())

            # ---- states ----
            Lh_pp = [sp.tile([128, NL], dt.bfloat16, name="Lh_a"),
                     sp.tile([128, NL], dt.bfloat16, name="Lh_b")]
            Ch = sp.tile([128, NCL], dt.bfloat16, name="Ch")
            Lc = sp.tile([128, NL], dt.float32, name="Lc")
            Cc = sp.tile([128, NCL], dt.float32, name="Cc")
            nc.scalar.dma_start(out=Lh_pp[0], in_=lh0.ap())
            nc.scalar.dma_start(out=Ch, in_=ch0.ap())
            nc.vector.memset(Lc, 0.0)
            nc.vector.memset(Cc, 0.0)

            # ---- resident B slices: 3 b1 slabs + source-cores 0-2 of b2
            # (48KB/part each); the rest streams per round, phase-balanced ----
            B1RES = ((0, 0), (1, 0), (2, 0))
            b1res = {}
            for nn, h in B1RES:
                b1res[nn, h] = cp.tile([128, 32 * 512], dt.float8e4,
                                       name=f"b1res{nn}{h}")
                nc.sync.dma_start(out=b1res[nn, h], in_=b1.ap()[nn, h])
            NRES2 = 3                  # resident source-cores of b2
            b2res = {}
            for nnl in range(2):
                b2res[nnl] = cp.tile([128, NRES2 * 2 * 4096], dt.float8e4,
                                     name=f"b2res{nnl}")
                nc.scalar.dma_start(
                    out=b2res[nnl].rearrange("p (k e) -> p k e", e=4096),
                    in_=b2.ap()[nnl][0:NRES2]
                    .rearrange("k g p e -> p (k g) e"))

            # ---- round-persistent work tiles ----
            lpre_full = wp.tile([128, NL_TOT], dt.float8e4, name="lpre_full")
            ag1_in = [dp.tile([128, 512], dt.float8e4, name=f"ag1_in{h}")
                      for h in range(2)]
            ag2_in = [dp.tile([128, 1024], dt.float8e4, name=f"ag2_in{g}")
                      for g in range(2)]

            def msg_mlp(which, nn, src_sl, Lh_src=None):
                """3-layer message MLP for one 512 chunk -> img tile + stage."""
                pfx = "lm" if which == "l" else "cm"
                h1 = kp.tile([128, 512], dt.bfloat16, tag="h1",
                             name=f"h1{which}_{nn}")
                h2 = kp.tile([128, 512], dt.bfloat16, tag="h2",
                             name=f"h2{which}_{nn}")
                img = kp.tile([128, 512], dt.float8e4, tag="img",
                              name=f"img{which}_{nn}")
                src = Lh_src[:, src_sl] if which == "l" else Ch[:, src_sl]
                ps = psm.tile([128, 512], dt.float32, tag="m",
                              name=f"{pfx}1_{nn}")
                nc.tensor.matmul(ps, C[f"{pfx}_w1t"], src, start=True, stop=True)
                nc.scalar.activation(h1, ps, AF.Relu, bias=C[f"{pfx}_b1"])
                ps = psm.tile([128, 512], dt.float32, tag="m",
                              name=f"{pfx}2_{nn}")
                nc.tensor.matmul(ps, C[f"{pfx}_w2t"], h1, start=True, stop=True)
                nc.scalar.activation(h2, ps, AF.Relu, bias=C[f"{pfx}_b2"])
                scn = "rowsc" if which == "l" else "colsc"
                for j in range(4):
                    ps = psm.tile([128, 128], dt.float32, tag="m",
                                  name=f"{pfx}3_{nn}_{j}")
                    nc.tensor.matmul(ps, h2[:, 128 * j:128 * (j + 1)],
                                     C[f"{pfx}_w3t"], start=True, stop=True)
                    nc.scalar.activation(img[:, 128 * j:128 * (j + 1)], ps,
                                         AF.Copy,
                                         scale=C[scn][:, 4 * nn + j:
                                                      4 * nn + j + 1])
                if which == "l":
                    nc.scalar.dma_start(out=ag1_in[nn], in_=img)
                    nc.gpsimd.collective_compute(
                        "AllGather", ALU.bypass, replica_groups=rg,
                        ins=[ag1_in[nn].opt()], outs=[ag1_out[nn].ap().opt()])
                else:
                    g = nn // 2
                    nc.scalar.dma_start(
                        out=ag2_in[g][:, 512 * (nn % 2):512 * (nn % 2 + 1)],
                        in_=img)
                    if nn % 2 == 1:
                        nc.gpsimd.collective_compute(
                            "AllGather", ALU.bypass, replica_groups=rg,
                            ins=[ag2_in[g].opt()],
                            outs=[ag2_out[g].ap().opt()])

            def land_ag1(h):
                # gathered group h -> lpre_full tiles t = 8kk + 4h + j
                dst = lpre_full.rearrange("p (k e) -> p k e", e=1024)[
                    :, :, 512 * h:512 * (h + 1)]
                nc.sync.dma_start(out=dst, in_=ag1_out[h].ap()
                                   .rearrange("k p e -> p k e"))

            def land_ag2(g, r):
                """Gathered clause group g -> two half buffers (cores 0-3,
                4-7; 32 clause-tiles each), landed off the collective
                queue so they fire the moment the mesh completes."""
                halves = []
                for hh in range(2):
                    cg = bp.tile([128, 32 * 128], dt.float8e4, tag="cgrp",
                                 bufs=2, name=f"cgrp_{r}_{g}_{hh}")
                    nc.sync.dma_start(
                        out=cg.rearrange("p (k e) -> p k e", e=1024),
                        in_=ag2_out[g].ap()[4 * hh:4 * (hh + 1)]
                        .rearrange("k p e -> p k e"))
                    halves.append(cg)
                return halves

            def evict(which, cc, acc_ps):
                """PSUM -> scaled+bias-corrected message chunk (DVE only)."""
                sl = slice(512 * cc, 512 * (cc + 1))
                scale = C["colb"] if which == "c" else C["rowb"]
                korr = C["korr_c"] if which == "c" else C["korr_l"]
                msg = kp.tile([128, 512], dt.bfloat16, tag="msg", bufs=2,
                              name=f"msg_{which}_{cc}")
                nc.vector.tensor_tensor(out=msg, in0=acc_ps,
                                        in1=scale[:, sl], op=ALU.mult)
                nc.vector.tensor_tensor(out=msg, in0=msg, in1=korr[:, sl],
                                        op=ALU.add)
                return msg

            def lstm_rest(which, cc, msg, Lh_src=None, Lh_dst=None):
                """LSTM gates + state update for one 512-col chunk."""
                sl = slice(512 * cc, 512 * (cc + 1))
                gts = []
                for g in range(4):
                    gs = slice(128 * g, 128 * (g + 1))
                    ps = psg.tile([128, 512], dt.float32, tag=f"g{g % 2}",
                                  name=f"ps_{which}_{cc}_{g}")
                    if which == "c":
                        nc.tensor.matmul(ps, C["cu_wt"][:, gs], msg,
                                         start=True, stop=False,
                                         skip_group_check=True)
                        nc.tensor.matmul(ps, C["cu_ut"][:, gs], Ch[:, sl],
                                         start=False, stop=True,
                                         skip_group_check=True)
                        bias = C["cu_b"][:, g:g + 1]
                    else:
                        flip_sl = slice(512 * (1 - cc), 512 * (2 - cc))
                        nc.tensor.matmul(ps, C["lu_wcl"][:, gs], msg,
                                         start=True, stop=False,
                                         skip_group_check=True)
                        nc.tensor.matmul(ps, C["lu_wfl"][:, gs],
                                         Lh_src[:, flip_sl],
                                         start=False, stop=False,
                                         skip_group_check=True)
                        nc.tensor.matmul(ps, C["lu_ut"][:, gs], Lh_src[:, sl],
                                         start=False, stop=True,
                                         skip_group_check=True)
                        bias = C["lu_b"][:, g:g + 1]
                    gt = kp.tile([128, 512], dt.bfloat16, tag=f"gate{g}",
                                 bufs=1, name=f"gt_{which}_{cc}_{g}")
                    nc.scalar.activation(gt, ps,
                                         AF.Tanh if g == 2 else AF.Sigmoid,
                                         bias=bias)
                    gts.append(gt)
                cell = Cc if which == "c" else Lc
                hout = Ch if which == "c" else Lh_dst
                t1 = kp.tile([128, 512], dt.float32, tag="t1", bufs=1,
                             name=f"t1_{which}_{cc}")
                t2 = kp.tile([128, 512], dt.bfloat16, tag="t2", bufs=1,
                             name=f"t2_{which}_{cc}")
                nc.vector.tensor_tensor(out=t1, in0=gts[1], in1=cell[:, sl],
                                        op=ALU.mult)
                nc.vector.tensor_tensor(out=t2, in0=gts[0], in1=gts[2],
                                        op=ALU.mult)
                nc.vector.tensor_tensor(out=cell[:, sl], in0=t1, in1=t2,
                                        op=ALU.add)
                t3 = kp.tile([128, 512], dt.float32, tag="t1", bufs=1,
                             name=f"t3_{which}_{cc}")
                nc.scalar.activation(t3, cell[:, sl], AF.Tanh)
                nc.vector.tensor_tensor(out=hout[:, sl], in0=gts[3], in1=t3,
                                        op=ALU.mult)

            def b1_prefetch(r):
                """Kick streamed-b1 half-slab DMAs for round r (sync queue)."""
                tiles = {}
                for h in range(2):
                    for nn in range(4):
                        if (nn, h) in b1res:
                            continue
                        for s2 in range(2):
                            t = bp.tile([128, 16 * 512], dt.float8e4,
                                        tag="b1s", bufs=2,
                                        name=f"b1s_{r}_{nn}_{h}_{s2}")
                            nc.sync.dma_start(
                                out=t, in_=b1.ap()[nn, h][:, 16 * 512 * s2:
                                                          16 * 512 * (s2 + 1)])
                            tiles[nn, h, s2] = t
                return tiles

            def mm1_block(nn, h, b1tiles, first, last):
                """16 DoubleRow MMs: dir-1 k-phase h for clause chunk nn."""
                for s2 in range(2):
                    bseg = (b1res[nn, h][:, 16 * 512 * s2:16 * 512 * (s2 + 1)]
                            if (nn, h) in b1res else b1tiles[nn, h, s2])
                    for ttp in range(8):
                        tt = 16 * s2 + 2 * ttp
                        t = 8 * (tt // 4) + 4 * h + tt % 4
                        lhsT = lpre_full[:, 128 * t:128 * (t + 2)] \
                            .rearrange("p (e d) -> p e d", e=2)
                        rhs = bseg[:, 1024 * ttp:1024 * (ttp + 1)] \
                            .rearrange("p (e c) -> p e c", e=2)
                        nc.tensor.matmul(
                            ps1[nn], lhsT, rhs,
                            start=(first and s2 == 0 and ttp == 0),
                            stop=(last and s2 == 1 and ttp == 7),
                            perf_mode=mybir.MatmulPerfMode.DoubleRow,
                            skip_group_check=True)

            def b2_prefetch(r):
                """Kick streamed-b2 DMAs for round r (scalar queue), one
                4KB tile per (clause half, lit chunk, source core), issued
                in exact consumption order."""
                tiles = {}
                for g in range(2):
                    for nnl in range(2):
                        for kk in range(NRES2, 8):
                            t = bp.tile([128, 4096], dt.float8e4,
                                        tag="b2s", bufs=4,
                                        name=f"b2s_{r}_{g}_{nnl}_{kk}")
                            nc.scalar.dma_start(out=t,
                                                in_=b2.ap()[nnl, kk, g])
                            tiles[g, nnl, kk] = t
                return tiles

            def mm2_block(nnl, g, halves, b2tiles, first, last):
                """32 DoubleRow MMs: dir-2 clause group g for lit chunk nnl.

                cgrp half hh = kk//4, local pair i = 2*(4*(kk%4) + q4)."""
                for kk in range(8):
                    off = 4096 * (2 * kk + g)
                    rhs_base = (b2res[nnl][:, off:off + 4096]
                                if kk < NRES2 else b2tiles[g, nnl, kk])
                    cg = halves[kk // 4]
                    for q4 in range(4):
                        i = 2 * (4 * (kk % 4) + q4)
                        lhsT = cg[:, 128 * i:128 * (i + 2)] \
                            .rearrange("p (e d) -> p e d", e=2)
                        rhs = rhs_base[:, 1024 * q4:1024 * (q4 + 1)] \
                            .rearrange("p (e c) -> p e c", e=2)
                        nc.tensor.matmul(
                            ps2[nnl], lhsT, rhs,
                            start=(first and kk == 0 and q4 == 0),
                            stop=(last and kk == 7 and q4 == 3),
                            perf_mode=mybir.MatmulPerfMode.DoubleRow,
                            skip_group_check=True)

            # ---- prologue: L message of round 0 (kicks AG1_0, AG1_1) ----
            for nn in range(2):
                msg_mlp("l", nn, slice(512 * nn, 512 * (nn + 1)),
                        Lh_src=Lh_pp[0])

            for r in range(rounds):
                Lh = Lh_pp[r % 2]
                Lh_new = Lh_pp[(r + 1) % 2]

                # ===== dir-1 (A^T @ Lpre), phase-major; C side staggered one
                # chunk behind so its serial ACT/DVE chain hides under the
                # next chunk's dense MMs =====
                ps1 = [psd.tile([128, 512], dt.float32, tag=f"d{nn}",
                                name=f"ps1_{r}_{nn}") for nn in range(4)]
                b1t = b1_prefetch(r)
                b2t = b2_prefetch(r)
                cmsgs = [None] * 4
                for h in range(2):
                    land_ag1(h)
                    if h == 0:
                        for nn in range(4):
                            mm1_block(nn, 0, b1t, first=True, last=False)
                    else:
                        for nn in range(4):
                            mm1_block(nn, 1, b1t, first=False, last=True)
                            cmsgs[nn] = evict("c", nn, ps1[nn])
                            if nn >= 1:
                                cs = nn - 1
                                lstm_rest("c", cs, cmsgs[cs])
                                msg_mlp("c", cs, slice(512 * cs, 512 * (cs + 1)))
                        for cs in (3,):
                            lstm_rest("c", cs, cmsgs[cs])
                            msg_mlp("c", cs, slice(512 * cs, 512 * (cs + 1)))

                # ===== dir-2 (A @ Cpre) group-major + L side staggered =====
                ps2 = [psd.tile([128, 512], dt.float32, tag=f"d{nnl}",
                                name=f"ps2_{r}_{nnl}") for nnl in range(2)]
                for g in range(2):
                    cgrp = land_ag2(g, r)
                    if g == 0:
                        mm2_block(0, 0, cgrp, b2t, first=True, last=False)
                        mm2_block(1, 0, cgrp, b2t, first=True, last=False)
                    else:
                        mm2_block(0, 1, cgrp, b2t, first=False, last=True)
                        lmsg0 = evict("l", 0, ps2[0])
                        mm2_block(1, 1, cgrp, b2t, first=False, last=True)
                        lstm_rest("l", 0, lmsg0, Lh_src=Lh, Lh_dst=Lh_new)
                        if r < rounds - 1:
                            msg_mlp("l", 0, slice(0, 512), Lh_src=Lh_new)
                        lmsg1 = evict("l", 1, ps2[1])
                        lstm_rest("l", 1, lmsg1, Lh_src=Lh, Lh_dst=Lh_new)
                        if r < rounds - 1:
                            msg_mlp("l", 1, slice(512, 1024), Lh_src=Lh_new)

            # ===== vote MLP (bias of last layer added host-side) =====
            Lh_fin = Lh_pp[rounds % 2]
            vote_sb = wp.tile([1, NL], dt.float32, name="vote_sb")
            for nn in range(2):
                sl = slice(512 * nn, 512 * (nn + 1))
                h1 = kp.tile([128, 512], dt.bfloat16, tag="h1", name=f"vh1_{nn}")
                h2 = kp.tile([128, 512], dt.bfloat16, tag="h2", name=f"vh2_{nn}")
                ps = psm.tile([128, 512], dt.float32, tag="m", name=f"v1_{nn}")
                nc.tensor.matmul(ps, C["lv_w1t"], Lh_fin[:, sl],
                                 start=True, stop=True)
                nc.scalar.activation(h1, ps, AF.Relu, bias=C["lv_b1"])
                ps = psm.tile([128, 512], dt.float32, tag="m", name=f"v2_{nn}")
                nc.tensor.matmul(ps, C["lv_w2t"], h1,
                                 start=True, stop=True)
                nc.scalar.activation(h2, ps, AF.Relu, bias=C["lv_b2"])
                ps = psm.tile([1, 512], dt.float32, tag="m", name=f"v3_{nn}")
                nc.tensor.matmul(ps, C["lv_w3t"], h2,
                                 start=True, stop=True)
                nc.scalar.activation(vote_sb[0:1, sl], ps, AF.Copy)
            nc.scalar.dma_start(out=vote_out.ap(), in_=vote_sb)

    nc.compile()
    return nc


# ---------------------------------------------------------------------------
# host-side input preparation
# ---------------------------------------------------------------------------

def prep_inputs(inputs):
    g = {k: np.asarray(v) for k, v in inputs.items()}
    lit_idx = g["lit_idx"].astype(np.int64)
    clause_idx = g["clause_idx"].astype(np.int64)

    B = np.zeros((NL_TOT, NCL_TOT), np.bool_)
    B[lit_idx, clause_idx] = True
    degc = B.sum(0).astype(np.float32)
    degl = B.sum(1).astype(np.float32)
    col = (np.float32(1.0) / (np.sqrt(degc) + np.float32(1e-6))).astype(np.float32)
    row = (np.float32(1.0) / (np.sqrt(degl) + np.float32(1e-6))).astype(np.float32)
    # degree-0 rows/cols of A are structurally zero: clamp their scales so the
    # gained fp8 messages stay finite (mathematically identical result)
    col = np.where(degc > 0, col, np.float32(0)).astype(np.float32)
    row = np.where(degl > 0, row, np.float32(0)).astype(np.float32)

    # permuted lit order: core k <- [512k..512k+512) u [4096+512k..4096+512k+512)
    lit_order = np.concatenate(
        [np.concatenate([np.arange(512 * k, 512 * (k + 1)),
                         NV + np.arange(512 * k, 512 * (k + 1))])
         for k in range(NCORES)])
    Bu = B.astype(np.uint8) * FP8_ONE
    Bp = Bu[lit_order]                      # [8192, 16384] permuted rows
    row_p = row[lit_order]

    Bf32 = B.astype(np.float32)
    s_c = row @ Bf32                        # [NCL_TOT]
    scol_full = (col * s_c).astype(np.float32)
    s_l = Bf32 @ col
    srow_full = ((row * s_l).astype(np.float32))[lit_order]

    lm_b3 = np.asarray(g["lm_b3"], np.float32)
    cm_b3 = np.asarray(g["cm_b3"], np.float32)

    def b(x):
        return np.ascontiguousarray(np.asarray(x, np.float32)).astype(bf16)

    common = {
        "lm_w1t": b(g["lm_w1"].T), "lm_w2t": b(g["lm_w2"].T), "lm_w3t": b(g["lm_w3"].T),
        "cm_w1t": b(g["cm_w1"].T), "cm_w2t": b(g["cm_w2"].T), "cm_w3t": b(g["cm_w3"].T),
        "lv_w1t": b(g["lv_w1"].T), "lv_w2t": b(g["lv_w2"].T), "lv_w3t": b(g["lv_w3"].T),
        "lm_b1": np.asarray(g["lm_b1"], np.float32).reshape(128, 1),
        "lm_b2": np.asarray(g["lm_b2"], np.float32).reshape(128, 1),
        "cm_b1": np.asarray(g["cm_b1"], np.float32).reshape(128, 1),
        "cm_b2": np.asarray(g["cm_b2"], np.float32).reshape(128, 1),
        "lv_b1": np.asarray(g["lv_b1"], np.float32).reshape(128, 1),
        "lv_b2": np.asarray(g["lv_b2"], np.float32).reshape(128, 1),
        "cu_wt": b(g["cu_wih"].T), "cu_ut": b(g["cu_whh"].T),
        "lu_wcl": b(g["lu_wih"][:, :D].T), "lu_wfl": b(g["lu_wih"][:, D:].T),
        "lu_ut": b(g["lu_whh"].T),
        "cu_b": np.asarray(g["cu_bih"] + g["cu_bhh"], np.float32).reshape(4, 128).T.copy(),
        "lu_b": np.asarray(g["lu_bih"] + g["lu_bhh"], np.float32).reshape(4, 128).T.copy(),
        "lh0": np.ascontiguousarray(np.broadcast_to(
            np.asarray(g["L_init_w"][:, 0] + g["L_init_b"], np.float32)[:, None],
            (128, NL))).astype(bf16),
        "ch0": np.ascontiguousarray(np.broadcast_to(
            np.asarray(g["C_init_w"][:, 0] + g["C_init_b"], np.float32)[:, None],
            (128, NCL))).astype(bf16),
    }

    in_maps = []
    for k in range(NCORES):
        lsl = slice(NL * k, NL * (k + 1))
        csl = slice(NCL * k, NCL * (k + 1))
        # b1: Bp[:, csl] is [t(64)*128p rows, nn(4)*512c cols]
        #     [kk(8), jh(2), jj(4), p, nn, c] -> [nn, jh, p, kk, jj, c]
        X = Bp[:, csl].reshape(8, 2, 4, 128, 4, 512)
        b1k = np.ascontiguousarray(X.transpose(4, 1, 3, 0, 2, 5)).reshape(
            4, 2, 128, 32 * 512).view(f8)
        # b2: Bp[lsl, :].T is [T(128)*128p clause rows, nnl(2)*512l cols]
        #     T = 16*kk + 8*g + 2*q4 + e
        Y = Bp[lsl, :].T.reshape(8, 2, 4, 2, 128, 2, 512)
        # axes: kk, g, q4, e, p, nnl, l -> nnl, kk, g, p, q4, e, l
        b2k = np.ascontiguousarray(Y.transpose(5, 0, 1, 4, 2, 3, 6)).reshape(
            2, 8, 2, 128, 4 * 1024).view(f8)
        m = dict(common)
        m.update({
            "b1": b1k,
            "b2": b2k,
            "korr_c": np.ascontiguousarray(
                lm_b3[:, None] * scol_full[None, csl]).astype(bf16),
            "korr_l": np.ascontiguousarray(
                cm_b3[:, None] * srow_full[None, lsl]).astype(bf16),
            "colb": np.ascontiguousarray(
                np.broadcast_to(col[csl][None, :] / GAIN, (128, NCL))).astype(bf16),
            "rowb": np.ascontiguousarray(
                np.broadcast_to(row_p[lsl][None, :] / GAIN, (128, NL))).astype(bf16),
            "rowsc": np.ascontiguousarray(
                GAIN * row_p[lsl].reshape(8, 128).T).astype(np.float32),
            "colsc": np.ascontiguousarray(
                GAIN * col[csl].reshape(16, 128).T).astype(np.float32),
        })
        in_maps.append(m)
    return in_maps


def selfcheck_layouts(in_maps, lit_idx, clause_idx):
    """Random probes: device-layout b1/b2 entries vs the raw B matrix."""
    B = np.zeros((NL_TOT, NCL_TOT), np.uint8)
    B[lit_idx, clause_idx] = FP8_ONE
    lit_order = np.concatenate(
        [np.concatenate([np.arange(512 * k, 512 * (k + 1)),
                         NV + np.arange(512 * k, 512 * (k + 1))])
         for k in range(NCORES)])
    Bp = B[lit_order]
    rng = np.random.default_rng(1)
    for k in (0, 3):
        b1k = in_maps[k]["b1"].view(np.uint8).reshape(4, 2, 128, 32, 512)
        for _ in range(50):
            nn, h, p, tt, c = (rng.integers(4), rng.integers(2), rng.integers(128),
                               rng.integers(32), rng.integers(512))
            t = 8 * (tt // 4) + 4 * h + tt % 4
            want = Bp[128 * t + p, 2048 * k + 512 * nn + c]
            assert b1k[nn, h, p, tt, c] == want, (k, nn, h, p, tt, c)
        b2k = in_maps[k]["b2"].view(np.uint8).reshape(2, 8, 2, 128, 4, 2, 512)
        for _ in range(80):
            nnl, kk, gg, p, q4, e, l = (
                rng.integers(2), rng.integers(8), rng.integers(2),
                rng.integers(128), rng.integers(4), rng.integers(2),
                rng.integers(512))
            T = 16 * kk + 8 * gg + 2 * q4 + e
            want = Bp[1024 * k + 512 * nnl + l, 128 * T + p]
            assert b2k[nnl, kk, gg, p, q4, e, l] == want, (k, nnl, kk, gg, p, q4, e, l)


_PROGRAM_CACHE = {}


def _get_program(rounds):
    if rounds not in _PROGRAM_CACHE:
        _PROGRAM_CACHE[rounds] = build_program(rounds)
    return _PROGRAM_CACHE[rounds]


def run_device(inputs, trace=False, rounds=None, **kw):
    if rounds is None:
        rounds = int(inputs.get("n_rounds", 16))
    in_maps = prep_inputs(inputs)
    nc = _get_program(rounds)
    res = bass_utils.run_bass_kernel_spmd(
        nc, in_maps, core_ids=list(range(NCORES)), trace=trace, **kw)
    return res


def assemble_votes(res_results, lv_b3):
    votes = np.stack([np.asarray(res_results[k]["vote"]).reshape(NL)
                      for k in range(NCORES)])   # [8, 1024]
    vote = votes + np.float32(lv_b3)
    pos = vote[:, :512].reshape(NV)              # var v -> core v//512
    neg = vote[:, 512:].reshape(NV)
    vj = np.stack([pos, neg], axis=1)            # [4096, 2]
    return vj.reshape(32, -1).mean(axis=1).astype(np.float32)


def kernel(**inputs) -> np.ndarray:
    res = run_device(inputs)
    return assemble_votes(res.results, np.asarray(inputs["lv_b3"]).reshape(-1)[0])


# revision 28
# speedup vs baseline: 1.0925x; 1.0559x over previous
"""NeuroSAT message-passing kernel for 8 Trainium2 NeuronCores (Bass/Tile).

Strategy
--------
The dense adjacency factors as A = D_row @ B @ D_col with B binary, so B
is carried in fp8 (1.0/0.0 exact in e4m3) as the *moving* matmul operand
against fp8 stationary message tiles (DoubleRow, K=256/instr); the degree
scalings are per-partition activation scales / free tensor_tensor
multiplies at PSUM eviction. The scaling-entangled final-layer MLP biases
are rank-1 (b3 outer scol/srow) and get added as precomputed outer-product
tiles on the (idle) Vector engine at eviction time.

Sharding (8 cores):
  - clauses: core k owns [2048k, 2048k+2048)
  - literals: core k owns [512k, 512k+512) u [4096+512k, 4096+512k+512)
    (a positive block and its negation block, so NeuroSAT's "flip" is a
    local slice swap instead of a cross-core exchange)

v3 (latency-stall rework, after v2's HBM rework):
  - 6 of the 8 b1 slabs (B[:, my clauses], lit-partitioned) live
    permanently in SBUF; only clause-chunk 3 of b1 plus all of b2 stream
    per round (~24MB/round vs 34 when streaming everything).
  - 4 collectives per round (2 lit-side, 2 clause-side AllGathers), each
    kicked the moment its half of the messages exists and consumed
    group-major on the other side, so the ~10us-floor CC ops pipeline
    under dense matmul work.
  - The serial LSTM/MLP chains (PSUM evict -> gates -> cell -> message
    MLP, mostly ACT/DVE latency) are emitted staggered one chunk behind
    the dense B-contraction so the in-order PE queue never waits on them;
    evictions are hoisted right after each accumulation closes. This keeps
    the PE continuously busy, which also keeps the HAM clock-gate at
    2.4GHz (idle gaps re-throttle it to 1.2GHz for ~3.4us).
"""
import sys

sys.path.insert(0, "/opt/trn_rl_repo")

import numpy as np
import ml_dtypes

import concourse.bass as bass
import concourse.mybir as mybir
import concourse.tile as tile
from concourse import bacc
from concourse import bass_utils

dt = mybir.dt
AF = mybir.ActivationFunctionType
ALU = mybir.AluOpType
bf16 = ml_dtypes.bfloat16
f8 = ml_dtypes.float8_e4m3

NCORES = 8
D = 128
NL_TOT, NCL_TOT, NV = 8192, 16384, 4096
NL = NL_TOT // NCORES      # 1024 lits per core
NCL = NCL_TOT // NCORES    # 2048 clauses per core
FP8_ONE = 0x38             # bit pattern of 1.0 in float8_e4m3
GAIN = np.float32(128.0)   # power-of-2 pre-scale keeping fp8 messages normal-range


# ---------------------------------------------------------------------------
# device program
# ---------------------------------------------------------------------------

def build_program(rounds: int):
    nc = bacc.Bacc("TRN2", target_bir_lowering=False, debug=False,
                   num_devices=NCORES)

    def inp(name, shape, dty):
        return nc.dram_tensor(name, list(shape), dty, kind="ExternalInput")

    # b1[nn, h]: slab of 32 k-tiles [128p, 32tt, 512c] covering clause chunk
    #            nn, lit-tile phase h (tt = 4k + jj, global tile t = 8k+4h+jj)
    # b2[nnl, kk, g]: sub-slab of 4 DR pairs [128p, 4q, 2, 512l]; lit
    #            out-chunk nnl, source core kk, clause half g. Pair (kk, g,
    #            q4) covers global clause tiles (16kk + 8g + 2*q4, +1).
    b1 = inp("b1", [4, 2, 128, 32 * 512], dt.float8e4)
    b2 = inp("b2", [2, 8, 2, 128, 4 * 1024], dt.float8e4)
    w = {}
    for p in ("lm", "cm", "lv"):
        for l in ("w1t", "w2t", "w3t"):
            shape = [128, 1] if (p, l) == ("lv", "w3t") else [128, 128]
            w[f"{p}_{l}"] = inp(f"{p}_{l}", shape, dt.bfloat16)
        for l in ("b1", "b2"):
            w[f"{p}_{l}"] = inp(f"{p}_{l}", [128, 1], dt.float32)
    cu_wt = inp("cu_wt", [128, 512], dt.bfloat16)      # cu_wih.T
    cu_ut = inp("cu_ut", [128, 512], dt.bfloat16)      # cu_whh.T
    cu_b = inp("cu_b", [128, 4], dt.float32)
    lu_wcl = inp("lu_wcl", [128, 512], dt.bfloat16)    # lu_wih[:, :128].T
    lu_wfl = inp("lu_wfl", [128, 512], dt.bfloat16)    # lu_wih[:, 128:].T
    lu_ut = inp("lu_ut", [128, 512], dt.bfloat16)      # lu_whh.T
    lu_b = inp("lu_b", [128, 4], dt.float32)
    korr_c = inp("korr_c", [128, NCL], dt.bfloat16)    # lm_b3 outer scol
    korr_l = inp("korr_l", [128, NL], dt.bfloat16)     # cm_b3 outer srow
    colb = inp("colb", [128, NCL], dt.bfloat16)        # col/GAIN bcast over parts
    rowb = inp("rowb", [128, NL], dt.bfloat16)         # row/GAIN bcast over parts
    rowsc = inp("rowsc", [128, 8], dt.float32)         # GAIN*row, per lit-tile col
    colsc = inp("colsc", [128, 16], dt.float32)        # GAIN*col, per clause-tile
    lh0 = inp("lh0", [128, NL], dt.bfloat16)
    ch0 = inp("ch0", [128, NCL], dt.bfloat16)

    vote_out = nc.dram_tensor("vote", [1, NL], dt.float32, kind="ExternalOutput")
    ag1_out = [nc.dram_tensor(f"ag1_out{h}", [NCORES, 128, 512], dt.float8e4,
                              addr_space="Shared") for h in range(2)]
    ag2_out = [nc.dram_tensor(f"ag2_out{g}", [NCORES, 128, 1024], dt.float8e4,
                              addr_space="Shared") for g in range(2)]
    rg = [list(range(NCORES))]

    with tile.TileContext(nc) as tc:
        with (
            tc.tile_pool(name="const", bufs=1) as cp,
            tc.tile_pool(name="state", bufs=1) as sp,
            tc.tile_pool(name="work", bufs=1) as wp,
            tc.tile_pool(name="chunk", bufs=2) as kp,
            tc.tile_pool(name="bstream", bufs=2) as bp,
            tc.tile_pool(name="psd", bufs=1, space="PSUM") as psd,
            tc.tile_pool(name="psg", bufs=1, space="PSUM") as psg,
            tc.tile_pool(name="psm", bufs=2, space="PSUM") as psm,
            tc.tile_pool(name="dram", bufs=1, space="DRAM") as dp,
        ):
            # ---- constants into SBUF ----
            C = {}
            for name, t in [
                ("cu_wt", cu_wt), ("cu_ut", cu_ut), ("lu_wcl", lu_wcl),
                ("lu_wfl", lu_wfl), ("lu_ut", lu_ut),
            ]:
                C[name] = cp.tile([128, 512], dt.bfloat16, name=name)
                nc.scalar.dma_start(out=C[name], in_=t.ap())
            for p in ("lm", "cm", "lv"):
                for l in ("w1t", "w2t", "w3t"):
                    shape = [128, 1] if (p, l) == ("lv", "w3t") else [128, 128]
                    C[f"{p}_{l}"] = cp.tile(shape, dt.bfloat16, name=f"{p}_{l}")
                    nc.scalar.dma_start(out=C[f"{p}_{l}"], in_=w[f"{p}_{l}"].ap())
                for l in ("b1", "b2"):
                    C[f"{p}_{l}"] = cp.tile([128, 1], dt.float32, name=f"{p}_{l}")
                    nc.scalar.dma_start(out=C[f"{p}_{l}"], in_=w[f"{p}_{l}"].ap())
            for name, t, shape, dty in [
                ("cu_b", cu_b, [128, 4], dt.float32),
                ("lu_b", lu_b, [128, 4], dt.float32),
                ("korr_c", korr_c, [128, NCL], dt.bfloat16),
                ("korr_l", korr_l, [128, NL], dt.bfloat16),
                ("colb", colb, [128, NCL], dt.bfloat16),
                ("rowb", rowb, [128, NL], dt.bfloat16),
                ("rowsc", rowsc, [128, 8], dt.float32),
                ("colsc", colsc, [128, 16], dt.float32),
            ]:
                C[name] = cp.tile(shape, dty, name=name)
                nc.scalar.dma_start(out=C[name], in_=t.ap())

            # ---- states ----
            Lh_pp = [sp.tile([128, NL], dt.bfloat16, name="Lh_a"),
                     sp.tile([128, NL], dt.bfloat16, name="Lh_b")]
            Ch = sp.tile([128, NCL], dt.bfloat16, name="Ch")
            Lc = sp.tile([128, NL], dt.float32, name="Lc")
            Cc = sp.tile([128, NCL], dt.float32, name="Cc")
            nc.scalar.dma_start(out=Lh_pp[0], in_=lh0.ap())
            nc.scalar.dma_start(out=Ch, in_=ch0.ap())
            nc.vector.memset(Lc, 0.0)
            nc.vector.memset(Cc, 0.0)

            # ---- resident B slices: 3 b1 slabs + source-cores 0-2 of b2
            # (48KB/part each); the rest streams per round, phase-balanced ----
            B1RES = ((0, 0), (1, 0), (2, 0))
            b1res = {}
            for nn, h in B1RES:
                b1res[nn, h] = cp.tile([128, 32 * 512], dt.float8e4,
                                       name=f"b1res{nn}{h}")
                nc.sync.dma_start(out=b1res[nn, h], in_=b1.ap()[nn, h])
            NRES2 = 3                  # resident source-cores of b2
            b2res = {}
            for nnl in range(2):
                b2res[nnl] = cp.tile([128, NRES2 * 2 * 4096], dt.float8e4,
                                     name=f"b2res{nnl}")
                nc.scalar.dma_start(
                    out=b2res[nnl].rearrange("p (k e) -> p k e", e=4096),
                    in_=b2.ap()[nnl][0:NRES2]
                    .rearrange("k g p e -> p (k g) e"))

            # ---- round-persistent work tiles ----
            lpre_full = wp.tile([128, NL_TOT], dt.float8e4, name="lpre_full")
            ag1_in = [dp.tile([128, 512], dt.float8e4, name=f"ag1_in{h}")
                      for h in range(2)]
            ag2_in = [dp.tile([128, 1024], dt.float8e4, name=f"ag2_in{g}")
                      for g in range(2)]
            live_cgrps = {}

            def msg_mlp(which, nn, src_sl, Lh_src=None):
                """3-layer message MLP for one 512 chunk -> img tile + stage."""
                pfx = "lm" if which == "l" else "cm"
                h1 = kp.tile([128, 512], dt.bfloat16, tag="h1",
                             name=f"h1{which}_{nn}")
                h2 = kp.tile([128, 512], dt.bfloat16, tag="h2",
                             name=f"h2{which}_{nn}")
                img = kp.tile([128, 512], dt.float8e4, tag="img",
                              name=f"img{which}_{nn}")
                src = Lh_src[:, src_sl] if which == "l" else Ch[:, src_sl]
                ps = psm.tile([128, 512], dt.float32, tag="m",
                              name=f"{pfx}1_{nn}")
                nc.tensor.matmul(ps, C[f"{pfx}_w1t"], src, start=True, stop=True)
                nc.scalar.activation(h1, ps, AF.Relu, bias=C[f"{pfx}_b1"])
                ps = psm.tile([128, 512], dt.float32, tag="m",
                              name=f"{pfx}2_{nn}")
                nc.tensor.matmul(ps, C[f"{pfx}_w2t"], h1, start=True, stop=True)
                nc.scalar.activation(h2, ps, AF.Relu, bias=C[f"{pfx}_b2"])
                scn = "rowsc" if which == "l" else "colsc"
                for j in range(4):
                    ps = psm.tile([128, 128], dt.float32, tag="m",
                                  name=f"{pfx}3_{nn}_{j}")
                    nc.tensor.matmul(ps, h2[:, 128 * j:128 * (j + 1)],
                                     C[f"{pfx}_w3t"], start=True, stop=True)
                    nc.scalar.activation(img[:, 128 * j:128 * (j + 1)], ps,
                                         AF.Copy,
                                         scale=C[scn][:, 4 * nn + j:
                                                      4 * nn + j + 1])
                if which == "l":
                    nc.scalar.dma_start(out=ag1_in[nn], in_=img)
                    nc.gpsimd.collective_compute(
                        "AllGather", ALU.bypass, replica_groups=rg,
                        ins=[ag1_in[nn].opt()], outs=[ag1_out[nn].ap().opt()])
                    land_ag1(nn)
                else:
                    g = nn // 2
                    nc.scalar.dma_start(
                        out=ag2_in[g][:, 512 * (nn % 2):512 * (nn % 2 + 1)],
                        in_=img)
                    if nn % 2 == 1:
                        nc.gpsimd.collective_compute(
                            "AllGather", ALU.bypass, replica_groups=rg,
                            ins=[ag2_in[g].opt()],
                            outs=[ag2_out[g].ap().opt()])
                        live_cgrps[g] = land_ag2(g)

            def land_ag1(h):
                # gathered group h -> lpre_full tiles t = 8kk + 4h + j
                dst = lpre_full.rearrange("p (k e) -> p k e", e=1024)[
                    :, :, 512 * h:512 * (h + 1)]
                nc.gpsimd.dma_start(out=dst, in_=ag1_out[h].ap()
                                    .rearrange("k p e -> p k e"))

            def land_ag2(g):
                """Gathered clause group g -> two half buffers (cores 0-3,
                4-7; 32 clause-tiles each), emitted directly behind the
                producing collective so they fire the moment it completes."""
                halves = []
                for hh in range(2):
                    cg = bp.tile([128, 32 * 128], dt.float8e4, tag="cgrp",
                                 bufs=3, name=f"cgrp_{g}_{hh}")
                    nc.gpsimd.dma_start(
                        out=cg.rearrange("p (k e) -> p k e", e=1024),
                        in_=ag2_out[g].ap()[4 * hh:4 * (hh + 1)]
                        .rearrange("k p e -> p k e"))
                    halves.append(cg)
                return halves

            def evict(which, cc, acc_ps):
                """PSUM -> scaled+bias-corrected message chunk (DVE only)."""
                sl = slice(512 * cc, 512 * (cc + 1))
                scale = C["colb"] if which == "c" else C["rowb"]
                korr = C["korr_c"] if which == "c" else C["korr_l"]
                msg = kp.tile([128, 512], dt.bfloat16, tag="msg", bufs=2,
                              name=f"msg_{which}_{cc}")
                nc.vector.tensor_tensor(out=msg, in0=acc_ps,
                                        in1=scale[:, sl], op=ALU.mult)
                nc.vector.tensor_tensor(out=msg, in0=msg, in1=korr[:, sl],
                                        op=ALU.add)
                return msg

            def lstm_rest(which, cc, msg, Lh_src=None, Lh_dst=None):
                """LSTM gates + state update for one 512-col chunk."""
                sl = slice(512 * cc, 512 * (cc + 1))
                gts = []
                for g in range(4):
                    gs = slice(128 * g, 128 * (g + 1))
                    ps = psg.tile([128, 512], dt.float32, tag=f"g{g % 2}",
                                  name=f"ps_{which}_{cc}_{g}")
                    if which == "c":
                        nc.tensor.matmul(ps, C["cu_wt"][:, gs], msg,
                                         start=True, stop=False,
                                         skip_group_check=True)
                        nc.tensor.matmul(ps, C["cu_ut"][:, gs], Ch[:, sl],
                                         start=False, stop=True,
                                         skip_group_check=True)
                        bias = C["cu_b"][:, g:g + 1]
                    else:
                        flip_sl = slice(512 * (1 - cc), 512 * (2 - cc))
                        nc.tensor.matmul(ps, C["lu_wcl"][:, gs], msg,
                                         start=True, stop=False,
                                         skip_group_check=True)
                        nc.tensor.matmul(ps, C["lu_wfl"][:, gs],
                                         Lh_src[:, flip_sl],
                                         start=False, stop=False,
                                         skip_group_check=True)
                        nc.tensor.matmul(ps, C["lu_ut"][:, gs], Lh_src[:, sl],
                                         start=False, stop=True,
                                         skip_group_check=True)
                        bias = C["lu_b"][:, g:g + 1]
                    gt = kp.tile([128, 512], dt.bfloat16, tag=f"gate{g}",
                                 bufs=1, name=f"gt_{which}_{cc}_{g}")
                    nc.scalar.activation(gt, ps,
                                         AF.Tanh if g == 2 else AF.Sigmoid,
                                         bias=bias)
                    gts.append(gt)
                cell = Cc if which == "c" else Lc
                hout = Ch if which == "c" else Lh_dst
                t1 = kp.tile([128, 512], dt.float32, tag="t1", bufs=1,
                             name=f"t1_{which}_{cc}")
                t2 = kp.tile([128, 512], dt.bfloat16, tag="t2", bufs=1,
                             name=f"t2_{which}_{cc}")
                nc.vector.tensor_tensor(out=t1, in0=gts[1], in1=cell[:, sl],
                                        op=ALU.mult)
                nc.vector.tensor_tensor(out=t2, in0=gts[0], in1=gts[2],
                                        op=ALU.mult)
                nc.vector.tensor_tensor(out=cell[:, sl], in0=t1, in1=t2,
                                        op=ALU.add)
                t3 = kp.tile([128, 512], dt.float32, tag="t1", bufs=1,
                             name=f"t3_{which}_{cc}")
                nc.scalar.activation(t3, cell[:, sl], AF.Tanh)
                nc.vector.tensor_tensor(out=hout[:, sl], in0=gts[3], in1=t3,
                                        op=ALU.mult)

            def b1_prefetch(r):
                """Kick streamed-b1 half-slab DMAs for round r (sync queue)."""
                tiles = {}
                for h in range(2):
                    for nn in range(4):
                        if (nn, h) in b1res:
                            continue
                        for s2 in range(2):
                            t = bp.tile([128, 16 * 512], dt.float8e4,
                                        tag="b1s", bufs=2,
                                        name=f"b1s_{r}_{nn}_{h}_{s2}")
                            nc.sync.dma_start(
                                out=t, in_=b1.ap()[nn, h][:, 16 * 512 * s2:
                                                          16 * 512 * (s2 + 1)])
                            tiles[nn, h, s2] = t
                return tiles

            def mm1_block(nn, h, b1tiles, first, last):
                """16 DoubleRow MMs: dir-1 k-phase h for clause chunk nn."""
                for s2 in range(2):
                    bseg = (b1res[nn, h][:, 16 * 512 * s2:16 * 512 * (s2 + 1)]
                            if (nn, h) in b1res else b1tiles[nn, h, s2])
                    for ttp in range(8):
                        tt = 16 * s2 + 2 * ttp
                        t = 8 * (tt // 4) + 4 * h + tt % 4
                        lhsT = lpre_full[:, 128 * t:128 * (t + 2)] \
                            .rearrange("p (e d) -> p e d", e=2)
                        rhs = bseg[:, 1024 * ttp:1024 * (ttp + 1)] \
                            .rearrange("p (e c) -> p e c", e=2)
                        nc.tensor.matmul(
                            ps1[nn], lhsT, rhs,
                            start=(first and s2 == 0 and ttp == 0),
                            stop=(last and s2 == 1 and ttp == 7),
                            perf_mode=mybir.MatmulPerfMode.DoubleRow,
                            skip_group_check=True)

            def b2_prefetch(r):
                """Kick streamed-b2 DMAs for round r (scalar queue), one
                4KB tile per (clause half, lit chunk, source core), issued
                in exact consumption order."""
                tiles = {}
                for g in range(2):
                    for nnl in range(2):
                        for kk in range(NRES2, 8):
                            t = bp.tile([128, 4096], dt.float8e4,
                                        tag="b2s", bufs=4,
                                        name=f"b2s_{r}_{g}_{nnl}_{kk}")
                            nc.scalar.dma_start(out=t,
                                                in_=b2.ap()[nnl, kk, g])
                            tiles[g, nnl, kk] = t
                return tiles

            def mm2_block(nnl, g, halves, b2tiles, first, last):
                """32 DoubleRow MMs: dir-2 clause group g for lit chunk nnl.

                cgrp half hh = kk//4, local pair i = 2*(4*(kk%4) + q4)."""
                for kk in range(8):
                    off = 4096 * (2 * kk + g)
                    rhs_base = (b2res[nnl][:, off:off + 4096]
                                if kk < NRES2 else b2tiles[g, nnl, kk])
                    cg = halves[kk // 4]
                    for q4 in range(4):
                        i = 2 * (4 * (kk % 4) + q4)
                        lhsT = cg[:, 128 * i:128 * (i + 2)] \
                            .rearrange("p (e d) -> p e d", e=2)
                        rhs = rhs_base[:, 1024 * q4:1024 * (q4 + 1)] \
                            .rearrange("p (e c) -> p e c", e=2)
                        nc.tensor.matmul(
                            ps2[nnl], lhsT, rhs,
                            start=(first and kk == 0 and q4 == 0),
                            stop=(last and kk == 7 and q4 == 3),
                            perf_mode=mybir.MatmulPerfMode.DoubleRow,
                            skip_group_check=True)

            # ---- prologue: L message of round 0 (kicks AG1_0, AG1_1) ----
            for nn in range(2):
                msg_mlp("l", nn, slice(512 * nn, 512 * (nn + 1)),
                        Lh_src=Lh_pp[0])

            for r in range(rounds):
                Lh = Lh_pp[r % 2]
                Lh_new = Lh_pp[(r + 1) % 2]

                # ===== dir-1 (A^T @ Lpre), phase-major; C side staggered one
                # chunk behind so its serial ACT/DVE chain hides under the
                # next chunk's dense MMs =====
                ps1 = [psd.tile([128, 512], dt.float32, tag=f"d{nn}",
                                name=f"ps1_{r}_{nn}") for nn in range(4)]
                b1t = b1_prefetch(r)
                b2t = b2_prefetch(r)
                cmsgs = [None] * 4
                for h in range(2):
                    if h == 0:
                        for nn in range(4):
                            mm1_block(nn, 0, b1t, first=True, last=False)
                    else:
                        for nn in range(4):
                            mm1_block(nn, 1, b1t, first=False, last=True)
                            cmsgs[nn] = evict("c", nn, ps1[nn])
                            if nn >= 1:
                                cs = nn - 1
                                lstm_rest("c", cs, cmsgs[cs])
                                msg_mlp("c", cs, slice(512 * cs, 512 * (cs + 1)))
                        for cs in (3,):
                            lstm_rest("c", cs, cmsgs[cs])
                            msg_mlp("c", cs, slice(512 * cs, 512 * (cs + 1)))

                # ===== dir-2 (A @ Cpre) group-major + L side staggered =====
                ps2 = [psd.tile([128, 512], dt.float32, tag=f"d{nnl}",
                                name=f"ps2_{r}_{nnl}") for nnl in range(2)]
                for g in range(2):
                    cgrp = live_cgrps[g]
                    if g == 0:
                        mm2_block(0, 0, cgrp, b2t, first=True, last=False)
                        mm2_block(1, 0, cgrp, b2t, first=True, last=False)
                    else:
                        mm2_block(0, 1, cgrp, b2t, first=False, last=True)
                        lmsg0 = evict("l", 0, ps2[0])
                        mm2_block(1, 1, cgrp, b2t, first=False, last=True)
                        lstm_rest("l", 0, lmsg0, Lh_src=Lh, Lh_dst=Lh_new)
                        if r < rounds - 1:
                            msg_mlp("l", 0, slice(0, 512), Lh_src=Lh_new)
                        lmsg1 = evict("l", 1, ps2[1])
                        lstm_rest("l", 1, lmsg1, Lh_src=Lh, Lh_dst=Lh_new)
                        if r < rounds - 1:
                            msg_mlp("l", 1, slice(512, 1024), Lh_src=Lh_new)

            # ===== vote MLP (bias of last layer added host-side) =====
            Lh_fin = Lh_pp[rounds % 2]
            vote_sb = wp.tile([1, NL], dt.float32, name="vote_sb")
            for nn in range(2):
                sl = slice(512 * nn, 512 * (nn + 1))
                h1 = kp.tile([128, 512], dt.bfloat16, tag="h1", name=f"vh1_{nn}")
                h2 = kp.tile([128, 512], dt.bfloat16, tag="h2", name=f"vh2_{nn}")
                ps = psm.tile([128, 512], dt.float32, tag="m", name=f"v1_{nn}")
                nc.tensor.matmul(ps, C["lv_w1t"], Lh_fin[:, sl],
                                 start=True, stop=True)
                nc.scalar.activation(h1, ps, AF.Relu, bias=C["lv_b1"])
                ps = psm.tile([128, 512], dt.float32, tag="m", name=f"v2_{nn}")
                nc.tensor.matmul(ps, C["lv_w2t"], h1,
                                 start=True, stop=True)
                nc.scalar.activation(h2, ps, AF.Relu, bias=C["lv_b2"])
                ps = psm.tile([1, 512], dt.float32, tag="m", name=f"v3_{nn}")
                nc.tensor.matmul(ps, C["lv_w3t"], h2,
                                 start=True, stop=True)
                nc.scalar.activation(vote_sb[0:1, sl], ps, AF.Copy)
            nc.scalar.dma_start(out=vote_out.ap(), in_=vote_sb)

    nc.compile()
    return nc


# ---------------------------------------------------------------------------
# host-side input preparation
# ---------------------------------------------------------------------------

def prep_inputs(inputs):
    g = {k: np.asarray(v) for k, v in inputs.items()}
    lit_idx = g["lit_idx"].astype(np.int64)
    clause_idx = g["clause_idx"].astype(np.int64)

    B = np.zeros((NL_TOT, NCL_TOT), np.bool_)
    B[lit_idx, clause_idx] = True
    degc = B.sum(0).astype(np.float32)
    degl = B.sum(1).astype(np.float32)
    col = (np.float32(1.0) / (np.sqrt(degc) + np.float32(1e-6))).astype(np.float32)
    row = (np.float32(1.0) / (np.sqrt(degl) + np.float32(1e-6))).astype(np.float32)
    # degree-0 rows/cols of A are structurally zero: clamp their scales so the
    # gained fp8 messages stay finite (mathematically identical result)
    col = np.where(degc > 0, col, np.float32(0)).astype(np.float32)
    row = np.where(degl > 0, row, np.float32(0)).astype(np.float32)

    # permuted lit order: core k <- [512k..512k+512) u [4096+512k..4096+512k+512)
    lit_order = np.concatenate(
        [np.concatenate([np.arange(512 * k, 512 * (k + 1)),
                         NV + np.arange(512 * k, 512 * (k + 1))])
         for k in range(NCORES)])
    Bu = B.astype(np.uint8) * FP8_ONE
    Bp = Bu[lit_order]                      # [8192, 16384] permuted rows
    row_p = row[lit_order]

    Bf32 = B.astype(np.float32)
    s_c = row @ Bf32                        # [NCL_TOT]
    scol_full = (col * s_c).astype(np.float32)
    s_l = Bf32 @ col
    srow_full = ((row * s_l).astype(np.float32))[lit_order]

    lm_b3 = np.asarray(g["lm_b3"], np.float32)
    cm_b3 = np.asarray(g["cm_b3"], np.float32)

    def b(x):
        return np.ascontiguousarray(np.asarray(x, np.float32)).astype(bf16)

    common = {
        "lm_w1t": b(g["lm_w1"].T), "lm_w2t": b(g["lm_w2"].T), "lm_w3t": b(g["lm_w3"].T),
        "cm_w1t": b(g["cm_w1"].T), "cm_w2t": b(g["cm_w2"].T), "cm_w3t": b(g["cm_w3"].T),
        "lv_w1t": b(g["lv_w1"].T), "lv_w2t": b(g["lv_w2"].T), "lv_w3t": b(g["lv_w3"].T),
        "lm_b1": np.asarray(g["lm_b1"], np.float32).reshape(128, 1),
        "lm_b2": np.asarray(g["lm_b2"], np.float32).reshape(128, 1),
        "cm_b1": np.asarray(g["cm_b1"], np.float32).reshape(128, 1),
        "cm_b2": np.asarray(g["cm_b2"], np.float32).reshape(128, 1),
        "lv_b1": np.asarray(g["lv_b1"], np.float32).reshape(128, 1),
        "lv_b2": np.asarray(g["lv_b2"], np.float32).reshape(128, 1),
        "cu_wt": b(g["cu_wih"].T), "cu_ut": b(g["cu_whh"].T),
        "lu_wcl": b(g["lu_wih"][:, :D].T), "lu_wfl": b(g["lu_wih"][:, D:].T),
        "lu_ut": b(g["lu_whh"].T),
        "cu_b": np.asarray(g["cu_bih"] + g["cu_bhh"], np.float32).reshape(4, 128).T.copy(),
        "lu_b": np.asarray(g["lu_bih"] + g["lu_bhh"], np.float32).reshape(4, 128).T.copy(),
        "lh0": np.ascontiguousarray(np.broadcast_to(
            np.asarray(g["L_init_w"][:, 0] + g["L_init_b"], np.float32)[:, None],
            (128, NL))).astype(bf16),
        "ch0": np.ascontiguousarray(np.broadcast_to(
            np.asarray(g["C_init_w"][:, 0] + g["C_init_b"], np.float32)[:, None],
            (128, NCL))).astype(bf16),
    }

    in_maps = []
    for k in range(NCORES):
        lsl = slice(NL * k, NL * (k + 1))
        csl = slice(NCL * k, NCL * (k + 1))
        # b1: Bp[:, csl] is [t(64)*128p rows, nn(4)*512c cols]
        #     [kk(8), jh(2), jj(4), p, nn, c] -> [nn, jh, p, kk, jj, c]
        X = Bp[:, csl].reshape(8, 2, 4, 128, 4, 512)
        b1k = np.ascontiguousarray(X.transpose(4, 1, 3, 0, 2, 5)).reshape(
            4, 2, 128, 32 * 512).view(f8)
        # b2: Bp[lsl, :].T is [T(128)*128p clause rows, nnl(2)*512l cols]
        #     T = 16*kk + 8*g + 2*q4 + e
        Y = Bp[lsl, :].T.reshape(8, 2, 4, 2, 128, 2, 512)
        # axes: kk, g, q4, e, p, nnl, l -> nnl, kk, g, p, q4, e, l
        b2k = np.ascontiguousarray(Y.transpose(5, 0, 1, 4, 2, 3, 6)).reshape(
            2, 8, 2, 128, 4 * 1024).view(f8)
        m = dict(common)
        m.update({
            "b1": b1k,
            "b2": b2k,
            "korr_c": np.ascontiguousarray(
                lm_b3[:, None] * scol_full[None, csl]).astype(bf16),
            "korr_l": np.ascontiguousarray(
                cm_b3[:, None] * srow_full[None, lsl]).astype(bf16),
            "colb": np.ascontiguousarray(
                np.broadcast_to(col[csl][None, :] / GAIN, (128, NCL))).astype(bf16),
            "rowb": np.ascontiguousarray(
                np.broadcast_to(row_p[lsl][None, :] / GAIN, (128, NL))).astype(bf16),
            "rowsc": np.ascontiguousarray(
                GAIN * row_p[lsl].reshape(8, 128).T).astype(np.float32),
            "colsc": np.ascontiguousarray(
                GAIN * col[csl].reshape(16, 128).T).astype(np.float32),
        })
        in_maps.append(m)
    return in_maps


def selfcheck_layouts(in_maps, lit_idx, clause_idx):
    """Random probes: device-layout b1/b2 entries vs the raw B matrix."""
    B = np.zeros((NL_TOT, NCL_TOT), np.uint8)
    B[lit_idx, clause_idx] = FP8_ONE
    lit_order = np.concatenate(
        [np.concatenate([np.arange(512 * k, 512 * (k + 1)),
                         NV + np.arange(512 * k, 512 * (k + 1))])
         for k in range(NCORES)])
    Bp = B[lit_order]
    rng = np.random.default_rng(1)
    for k in (0, 3):
        b1k = in_maps[k]["b1"].view(np.uint8).reshape(4, 2, 128, 32, 512)
        for _ in range(50):
            nn, h, p, tt, c = (rng.integers(4), rng.integers(2), rng.integers(128),
                               rng.integers(32), rng.integers(512))
            t = 8 * (tt // 4) + 4 * h + tt % 4
            want = Bp[128 * t + p, 2048 * k + 512 * nn + c]
            assert b1k[nn, h, p, tt, c] == want, (k, nn, h, p, tt, c)
        b2k = in_maps[k]["b2"].view(np.uint8).reshape(2, 8, 2, 128, 4, 2, 512)
        for _ in range(80):
            nnl, kk, gg, p, q4, e, l = (
                rng.integers(2), rng.integers(8), rng.integers(2),
                rng.integers(128), rng.integers(4), rng.integers(2),
                rng.integers(512))
            T = 16 * kk + 8 * gg + 2 * q4 + e
            want = Bp[1024 * k + 512 * nnl + l, 128 * T + p]
            assert b2k[nnl, kk, gg, p, q4, e, l] == want, (k, nnl, kk, gg, p, q4, e, l)


_PROGRAM_CACHE = {}


def _get_program(rounds):
    if rounds not in _PROGRAM_CACHE:
        _PROGRAM_CACHE[rounds] = build_program(rounds)
    return _PROGRAM_CACHE[rounds]


def run_device(inputs, trace=False, rounds=None, **kw):
    if rounds is None:
        rounds = int(inputs.get("n_rounds", 16))
    in_maps = prep_inputs(inputs)
    nc = _get_program(rounds)
    res = bass_utils.run_bass_kernel_spmd(
        nc, in_maps, core_ids=list(range(NCORES)), trace=trace, **kw)
    return res


def assemble_votes(res_results, lv_b3):
    votes = np.stack([np.asarray(res_results[k]["vote"]).reshape(NL)
                      for k in range(NCORES)])   # [8, 1024]
    vote = votes + np.float32(lv_b3)
    pos = vote[:, :512].reshape(NV)              # var v -> core v//512
    neg = vote[:, 512:].reshape(NV)
    vj = np.stack([pos, neg], axis=1)            # [4096, 2]
    return vj.reshape(32, -1).mean(axis=1).astype(np.float32)


def kernel(**inputs) -> np.ndarray:
    res = run_device(inputs)
    return assemble_votes(res.results, np.asarray(inputs["lv_b3"]).reshape(-1)[0])
